# revision 100
# baseline (speedup 1.0000x reference)
"""GCN message-passing kernel for Trainium2, 8 NeuronCores (SPMD).

Strategy (graph-parallel, fp8 messages):
- Nodes are protein-contiguous, sharded across 8 cores at protein boundaries
  (16 proteins/core, padded to 6400 nodes/core). Within a core, nodes are
  bin-packed into 50 blocks of 128 balancing incoming-edge counts; slot
  s = blk*128 + pos maps to partition pos, chunk blk everywhere (h, dis,
  table, S, pooling) so aggregation blocks coincide with node chunks.
- Message table is fp8e4m3 at 256B row stride with a 128B payload
  (row = dis[src] * (h @ W)[src]); gathers use elem_size=128/elem_step=256
  (bass's %256 payload assert is bypassed via direct InstDMAGatherAnt
  construction - verified byte-exact on hardware), halving per-edge DMA
  cost vs bf16. Only real edges are gathered: the self-loop term is an
  identity matmul on the resident staging chunk, and the conv bias is a
  per-block diag(1/(16 dis)) @ (16 conv_b) matmul - both accumulate into
  the same PSUM group, so the gather stream is edges-only (NT=12 tiles
  per block instead of 13).
- Aggregation is node-major: acc[dst,feat] = S^T @ msgs with S a 0/1
  one-hot (exact in fp8) via DoubleRow fp8 matmuls (256 slots each);
  dis[dst] is applied EXACTLY by the relu epilogue's per-partition ACT
  scale. S rows are pure one-hots, so the first SGEN_BLOCKS blocks of S
  are generated on-chip (one DVE is_equal per block against an iota row,
  from a compact per-slot dst-position table) and only the rest load from
  DRAM - the DMA engines are the global bottleneck, DVE has slack.
- Pipeline: hi-stream gather chunks carry no dependency on the measured
  program's table write, so ring-depth many are prefetched to cover each
  layer's table-write + first-lo-descgen window; the table is written in
  halves so early wcasts stream out under the previous layer's gathers.
- h is kept bf16 both node-major (epilogue output; feeds readout) and
  feature-major (one PE transpose per chunk per layer; copies on Act
  while DVE generates S, on DVE afterwards).
- Readout is per-block in the last layer: scores via DVE mul+reduce
  against a broadcast att_w row, exp/mask immediately (denominator folded
  in at the end - softmax shift is skipped since scores are O(0.1)), and
  three accumulating pool matmuls (mean with host-folded 1/(n sqrt n),
  attention numerator, denominator); the final combine, transpose and
  projection run once per core after the block loop.
"""
import bisect
import os
import numpy as np
import ml_dtypes

DBG_LAYERS = int(os.environ.get("GCN_DBG_LAYERS", "4"))
DBG_NO_COLL = os.environ.get("GCN_DBG_NO_COLL", "") == "1"
DBG_NO_GATHER = os.environ.get("GCN_DBG_NO_GATHER", "") == "1"
DBG_NO_READOUT = os.environ.get("GCN_DBG_NO_READOUT", "") == "1"
DBG_DUMP_H = os.environ.get("GCN_DBG_DUMP_H", "") == "1"
DBG_DUMP_TB = os.environ.get("GCN_DBG_DUMP_TB", "") == "1"

import concourse.bacc as bacc
import concourse.tile as tile
import concourse.tile_utils as tile_utils
from concourse import mybir
from concourse.bass_utils import run_bass_kernel_spmd
from concourse.masks import make_identity

bf16 = ml_dtypes.bfloat16
E4 = ml_dtypes.float8_e4m3
AF = mybir.ActivationFunctionType

NC = 8
D = 128
L = 4
B = 128
PPC = B // NC          # proteins per core
NPAD = 6400            # padded nodes per core
NPADG = NC * NPAD      # global padded rows
NTB = NPAD // 128      # 50 chunks of 128 nodes == aggregation blocks
NBLK = NTB
LO_BOUND = 32000       # lo gather covers rows [0, 32000)
HI_BASE = 18560        # hi gather covers rows [18560, 51200): 32639 <= int16
GCH = 8192             # gather slots per dma_gather instruction (64 cols)
SGEN_BLOCKS = 28       # S blocks generated on-chip; the rest load from DRAM

f32 = mybir.dt.float32
bft = mybir.dt.bfloat16
fp8 = mybir.dt.float8e4
i16 = mybir.dt.int16


# ---------------------------------------------------------------- host prep

def _pack_idx(vals, slots):
    """int16 gather index layout: position i -> partition i%16, col i//16,
    replicated across the 128 partitions."""
    assert len(vals) == slots and slots % 16 == 0
    arr = np.asarray(vals, np.int16).reshape(slots // 16, 16).T  # [16, s//16]
    return np.ascontiguousarray(np.tile(arr, (8, 1)))


def _ceil128(x):
    return max(1, int(np.ceil(x / 128)))


def _host_prep(x, edge_index, batch, lysine_mask):
    N = x.shape[0]
    src = np.asarray(edge_index[0], np.int64)
    dst = np.asarray(edge_index[1], np.int64)
    batch = np.asarray(batch, np.int64)

    pcounts = np.bincount(batch, minlength=B)
    pstart = np.concatenate([[0], np.cumsum(pcounts)])
    cstart = pstart[np.arange(NC) * PPC]
    cend = pstart[(np.arange(NC) + 1) * PPC]
    ncore = cend - cstart
    assert ncore.max() <= NPAD - 1, f"core node count {ncore.max()} > {NPAD-1}"
    assert pcounts.max() <= 128 * NTB

    deg = np.bincount(dst, minlength=N).astype(np.float64) + 1.0
    dis = (1.0 / np.sqrt(deg)).astype(np.float32)
    core_of = np.searchsorted(cend, np.arange(N), side="right")

    # --- per-core node packing into NBLK blocks of 128, balancing in-slot
    # (in-edges + self) counts per block; (pos 127, blk 49) is reserved.
    blk = np.zeros(N, np.int64)
    pos = np.zeros(N, np.int64)
    for c in range(NC):
        nodes = np.arange(cstart[c], cend[c])
        tot = deg[nodes]
        order = np.argsort(-tot, kind="stable")
        caps = np.full(NBLK, 128, np.int64)
        caps[NBLK - 1] = 127
        loads = np.zeros(NBLK)
        cnts = np.zeros(NBLK, np.int64)
        for i in order:
            masked = np.where(cnts < caps, loads, np.inf)
            b = int(np.argmin(masked))
            blk[nodes[i]] = b
            # (pos 0, blk 49) is the reserved bias slot on every core
            pos[nodes[i]] = cnts[b] + (1 if b == NBLK - 1 else 0)
            cnts[b] += 1
            loads[b] += tot[i]
    slot = blk * 128 + pos                    # local pi slot
    grow = core_of * NPAD + pos * NTB + blk   # global table row

    # --- edge list: real edges only. Self-loops are applied on-chip via an
    # identity matmul on the staging chunk; the conv bias enters via a
    # per-block diagonal matmul (diag(1/(16 dis)) @ 16*conv_b), so there are
    # no pseudo-edges and every S row is a pure one-hot — which lets S be
    # GENERATED on-chip (DVE is_equal against an iota row) from a compact
    # per-slot dst-position table (rowdst) instead of a 10MB fp8 load.
    e_row = grow[src]
    e_core = core_of[dst]
    e_blk = blk[dst]
    e_col = pos[dst]

    cls = np.where(e_row < HI_BASE, 0,
                   np.where(e_row < LO_BOUND, 1, 2))
    key = e_core * NBLK + e_blk
    nl0 = np.bincount(key[cls == 0], minlength=NC * NBLK)
    nf = np.bincount(key[cls == 1], minlength=NC * NBLK)
    tot_cb = np.bincount(key, minlength=NC * NBLK)

    best = None
    for LO_T in range(_ceil128(nl0.max()), _ceil128(nl0.max()) + 4):
        lo_fill = np.minimum(LO_T * 128, nl0 + nf)
        HI_T = _ceil128((tot_cb - lo_fill).max())
        if best is None or LO_T + HI_T < best[0] + best[1]:
            best = (LO_T, HI_T)
    LO_T, HI_T = best
    NT = LO_T + HI_T

    iota_row = np.tile(np.arange(128, dtype=np.float32), (128, 1))
    per_core = []
    for c in range(NC):
        m = e_core == c
        rows_e, blk_e, col_e, cls_e = (
            e_row[m], e_blk[m], e_col[m], cls[m])
        order = np.lexsort((col_e, cls_e, blk_e))
        rows_e, blk_e, col_e, cls_e = (
            rows_e[order], blk_e[order], col_e[order], cls_e[order])
        bstart = np.searchsorted(blk_e, np.arange(NBLK))
        bend = np.searchsorted(blk_e, np.arange(NBLK), side="right")

        nodes = np.arange(cstart[c], cend[c])
        # dis / inv-dis in pi layout (pads -> 1 / 0)
        dis_nm = np.ones((128, NTB), np.float32)
        inv_nm = np.zeros((128, NTB), np.float32)
        dis_nm[pos[nodes], blk[nodes]] = dis[nodes]
        inv_nm[pos[nodes], blk[nodes]] = 1.0 / dis[nodes]

        lo_idx = np.zeros(NBLK * LO_T * 128, np.int64)
        hi_idx = np.zeros(NBLK * HI_T * 128, np.int64)  # already HI_BASE-offset
        # per-slot dst position, 255 for padding (never matches iota 0..127)
        rowdst = np.full((128, NBLK * NT), 255.0, np.float32)
        s_all = np.zeros((128, NBLK * NT * 128), np.float32)
        for b in range(NBLK):
            sl = slice(bstart[b], bend[b])
            r_b, c_b, k_b = rows_e[sl], col_e[sl], cls_e[sl]
            n = len(r_b)
            n0 = int((k_b == 0).sum())
            nfb = int((k_b == 1).sum())
            take = min(LO_T * 128 - n0, nfb)
            assert take >= 0, f"block lo overflow {n0} > {LO_T*128}"
            nlo = n0 + take
            nhi = n - nlo
            assert nhi <= HI_T * 128
            for stream, cnt, off, idxarr, base_t, ibase in (
                (0, nlo, 0, lo_idx, 0, 0),
                (1, nhi, nlo, hi_idx, LO_T, HI_BASE),
            ):
                if cnt == 0:
                    continue
                rr = r_b[off:off + cnt] - ibase
                cc = c_b[off:off + cnt]
                T = LO_T if stream == 0 else HI_T
                idxarr[b * T * 128: b * T * 128 + cnt] = rr
                k = np.arange(cnt)
                p = k % 128
                t = base_t + k // 128
                rowdst[p, b * NT + t] = cc
                s_all[p, (b * NT + t) * 128 + cc] = 1.0

        x_t = np.zeros((D, NPAD), np.float32)
        x_t[:, slot[nodes]] = np.asarray(x[nodes], np.float32).T

        lens = pcounts[c * PPC:(c + 1) * PPC]
        starts = np.concatenate([[0], np.cumsum(lens)])[:-1]
        q = np.arange(ncore[c])
        pj = np.searchsorted(starts, q, side="right") - 1
        pone = np.zeros((128, NTB * PPC), bf16)
        pone[pos[nodes], blk[nodes] * PPC + pj] = 1.0
        lens_f = np.asarray(lens, np.float64)
        cj = (1.0 / (np.maximum(lens_f, 1.0) * np.sqrt(lens_f + 1e-6)))
        pone_cj = (pone.astype(np.float32)
                   * np.tile(cj, NTB)[None, :]).astype(bf16)
        lys_nm = np.zeros((128, NTB), np.float32)
        lys_nm[pos[nodes], blk[nodes]] = np.asarray(
            lysine_mask[nodes], np.float32)

        per_core.append(dict(
            x_t=x_t.astype(bf16),
            rowdst=rowdst,
            iota_row=iota_row,
            inv16=inv_nm / 16.0,
            s_part=np.ascontiguousarray(
                s_all[:, SGEN_BLOCKS * NT * 128:]).astype(E4),
            idx_lo=_pack_idx(lo_idx, NBLK * LO_T * 128),
            idx_hi=_pack_idx(hi_idx, NBLK * HI_T * 128),
            dis_nm=dis_nm,
            pone=pone,
            pone_cj=pone_cj,
            lys_nm=lys_nm,
        ))
    return per_core, LO_T, HI_T, NT


# ---------------------------------------------------------------- program

def _dma_gather_128(nc, out_ap, in_ap, idxs_ap, num_idxs):
    """dma_gather with a 128B payload on a 256B-stride table (elem_size=128
    fp8, elem_step=256). Bypasses bass's %256 payload assert; verified
    byte-exact on hardware."""
    g = nc.gpsimd
    _in_ap = g.lower_ap_dma(in_ap, for_custom_bir_dma=True)
    _idxs_ap = g.lower_ap(idxs_ap)
    _out_ap = g.lower_ap(out_ap)
    return g.add_instruction(mybir.InstDMAGatherAnt(
        name=g.bass.get_next_instruction_name(),
        ins=[*_in_ap, _idxs_ap, g.lower_val_access(g.to_reg(num_idxs))],
        outs=[_out_ap],
        transpose=False, num_idxs=num_idxs, elem_size=128,
        stride_bytes_256=1, gen_mode=0, single_packet=False,
        queue_num=0, sbuf_tokens_per_rank=0, sbuf_free_dim_per_rank=0,
        sbuf_free_dim_pad_per_rank=0, sbuf_byte_offset=0))


def _build_program(LO_T, HI_T, NT):
    tile_utils.max_sbuf_usage = 208 * 1024
    nc = bacc.Bacc("TRN2", target_bir_lowering=False, num_devices=NC,
                   num_swdge_queues=2)

    din = {}
    for name, shape, dt in [
        ("x_t", [D, NPAD], bft),
        ("rowdst", [128, NBLK * NT], f32),
        ("iota_row", [128, 128], f32),
        ("inv16", [128, NTB], f32),
        ("s_part", [128, (NBLK - SGEN_BLOCKS) * NT * 128], fp8),
        ("idx_lo", [128, NBLK * LO_T * 8], i16),
        ("idx_hi", [128, NBLK * HI_T * 8], i16),
        ("dis_nm", [128, NTB], f32),
        ("pone", [128, NTB * PPC], bft),
        ("pone_cj", [128, NTB * PPC], bft),
        ("lys_nm", [128, NTB], f32),
        ("convw", [D, L * D], bft),
        ("convb_pre", [128, L * D], fp8),
        ("attw_row", [1, D], f32),
        ("outw", [D, 64], f32),
        ("outb", [64, 1], f32),
    ]:
        din[name] = nc.dram_tensor(name, shape, dt, kind="ExternalInput")
    out_t = nc.dram_tensor("out_t", [64, PPC], f32, kind="ExternalOutput")
    out_h = None
    if DBG_DUMP_H:
        out_h = nc.dram_tensor("out_h", [128, NPAD], bft,
                               kind="ExternalOutput")
    out_tb = None
    if DBG_DUMP_TB:
        out_tb = nc.dram_tensor("out_tb", [128, NTB * 128], fp8,
                                kind="ExternalOutput")

    LO_SLOTS = NBLK * LO_T * 128
    HI_SLOTS = NBLK * HI_T * 128

    with tile.TileContext(nc) as tc:
        with (
            tc.tile_pool(name="glob", bufs=1) as gp,
            tc.tile_pool(name="dram", bufs=1, space="DRAM") as dram,
            tc.tile_pool(name="msgs", bufs=4) as mp,
            tc.tile_pool(name="r2", bufs=2) as rp2,
            tc.tile_pool(name="ps_w", bufs=3, space="PSUM") as ps_w,
            tc.tile_pool(name="ps_agg", bufs=2, space="PSUM") as ps_agg,
            tc.tile_pool(name="ps_tr", bufs=1, space="PSUM") as ps_tr,
            tc.tile_pool(name="ps_r", bufs=1, space="PSUM") as ps_r,
            tc.tile_pool(name="ps_p", bufs=1, space="PSUM") as ps_p,
        ):
            # resident SBUF state
            h_fm = gp.tile([D, NPAD], bft, name="h_fm")
            nc.sync.dma_start(h_fm[:], din["x_t"][:])
            h_nm = gp.tile([128, NTB, 128], bft, name="h_nm")
            staging = gp.tile([128, NTB, 128], fp8, name="staging")
            s_sb = gp.tile([128, NBLK * NT * 128], fp8, name="s_sb")
            # idx_hi loads first (it gates the first hi-gather desc-gen),
            # then the wcast deps (convw/dis/convb), then the rest
            idx_hi = gp.tile([128, HI_SLOTS // 16], i16)
            nc.sync.dma_start(idx_hi[:], din["idx_hi"][:])
            convw = gp.tile([D, L * D], bft)
            nc.sync.dma_start(convw[:], din["convw"][:])
            dis_nm = gp.tile([128, NTB], f32)
            nc.sync.dma_start(dis_nm[:], din["dis_nm"][:])
            convb_pre = gp.tile([128, L * D], fp8)
            nc.sync.dma_start(convb_pre[:], din["convb_pre"][:])
            idx_lo = gp.tile([128, LO_SLOTS // 16], i16)
            nc.sync.dma_start(idx_lo[:], din["idx_lo"][:])
            rowdst = gp.tile([128, NBLK * NT], f32)
            nc.sync.dma_start(rowdst[:], din["rowdst"][:])
            iota_row = gp.tile([128, 128], f32)
            nc.sync.dma_start(iota_row[:], din["iota_row"][:])
            inv16 = gp.tile([128, NTB], f32)
            nc.sync.dma_start(inv16[:], din["inv16"][:])
            pone = gp.tile([128, NTB * PPC], bft)
            nc.sync.dma_start(pone[:], din["pone"][:])
            lys_nm = gp.tile([128, NTB], f32)
            nc.sync.dma_start(lys_nm[:], din["lys_nm"][:])
            pone_cj = gp.tile([128, NTB * PPC], bft)
            nc.sync.dma_start(pone_cj[:], din["pone_cj"][:])
            attw = gp.tile([1, D], f32)
            nc.sync.dma_start(attw[:], din["attw_row"][:])
            outw = gp.tile([D, 64], f32)
            nc.sync.dma_start(outw[:], din["outw"][:])
            outb = gp.tile([64, 1], f32)
            nc.sync.dma_start(outb[:], din["outb"][:])
            stripe = dram.tile([NPAD, 256], fp8)
            hws_full = dram.tile([NPADG, 256], fp8)
            tident = gp.tile([128, 128], bft)
            make_identity(nc, tident[:])
            ident = gp.tile([128, 128], f32)
            make_identity(nc, ident[:])
            ident8 = gp.tile([128, 128], fp8)
            make_identity(nc, ident8[:])
            ones_r = gp.tile([1, 128], f32)
            nc.vector.memset(ones_r[:], 1.0)
            ones_f = gp.tile([128, 1], f32)
            nc.vector.memset(ones_f[:], 1.0)
            ones_bf = gp.tile([128, 1], bft)
            nc.vector.memset(ones_bf[:], 1.0)

            # att_w broadcast to all partitions (ones outer product)
            psat = ps_r.tile([128, D], f32, tag="tr")
            nc.tensor.matmul(out=psat[:], lhsT=ones_r[:],
                             rhs=attw[:], start=True, stop=True)
            attrep = gp.tile([128, D], bft)
            nc.vector.tensor_copy(attrep[:], psat[:])
            sc_nm = gp.tile([128, NTB], f32)
            exl = gp.tile([128, NTB], f32)
            pex_all = gp.tile([128, NTB, PPC], bft)

            # per-block diag(1/(16 dis[dst])) in fp8: the bias matmul's lhsT
            # (same quantization as the old in-S bias column); filled
            # per-block alongside the S generation / load below
            diag8 = gp.tile([128, NTB, 128], fp8)

            def emit_sgen(b):
                # S block b: one-hot rows from per-slot dst positions via a
                # single DVE is_equal (iota row tiled along tiles; rowdst
                # broadcast along the 128 dst columns). Padding slots carry
                # 255 and generate all-zero rows. ~1.66us per block on DVE,
                # which only keeps ahead of the layer-0 aggregation pace for
                # the first SGEN_BLOCKS blocks — the rest load from DRAM.
                nc.vector.tensor_tensor(
                    out=s_sb[:, b * NT * 128:(b + 1) * NT * 128].rearrange(
                        "p (t c) -> p t c", t=NT),
                    in0=iota_row[:].rearrange(
                        "p (o c) -> p o c", o=1).broadcast_to([128, NT, 128]),
                    in1=rowdst[:, b * NT:(b + 1) * NT].rearrange(
                        "p (t o) -> p t o", o=1).broadcast_to([128, NT, 128]),
                    op=mybir.AluOpType.is_equal)
                nc.vector.tensor_scalar_mul(
                    diag8[:, b, :], ident8[:], inv16[:, b:b + 1])

            def emit_wcast(layer, b):
                # table chunk: staging[:, b, :] = fp8(dis * (h @ W)),
                # node-major via out = h_fm_chunk^T @ W
                pw = ps_w.tile([128, D], f32, tag="wmm")
                nc.tensor.matmul(
                    out=pw[:],
                    lhsT=h_fm[:, b * 128:(b + 1) * 128],
                    rhs=convw[:, layer * D:(layer + 1) * D],
                    start=True, stop=True)
                nc.scalar.activation(
                    staging[:, b, :], pw[:], AF.Copy,
                    scale=dis_nm[:, b:b + 1])

            SGEN_AHEAD = 10
            for b in range(NTB):
                emit_wcast(0, b)
            for b in range(min(SGEN_AHEAD, SGEN_BLOCKS)):
                emit_sgen(b)
            for b in range(SGEN_BLOCKS, NBLK):
                nc.vector.tensor_scalar_mul(
                    diag8[:, b, :], ident8[:], inv16[:, b:b + 1])
            # S for the later half of the loaded blocks arrives early on the
            # Act queue; the first piece is issued behind layer 0's table
            # write on the SP queue, where it fills the DMA engines during
            # the first lo gather's desc-gen window
            SG0 = SGEN_BLOCKS * NT * 128
            SGM = ((SGEN_BLOCKS + NBLK) // 2) * NT * 128
            nc.scalar.dma_start(s_sb[:, SGM:],
                                din["s_part"][:, SGM - SG0:])

            pall_mean = None
            pall_att = None
            for layer in range(DBG_LAYERS):
                last = layer == DBG_LAYERS - 1
                # table write in pieces so early-chunk payloads stream out
                # while later wcasts still run — the next layer's lo gathers
                # wait on all of them
                if DBG_NO_COLL:
                    # collective stand-in: write the staged payload straight
                    # into this core's own region of the table (same local
                    # DMA work as the real path's stripe write)
                    tpm = hws_full[0:NPAD, 0:128].rearrange(
                        "(p k) f -> p k f", k=NTB)
                else:
                    tpm = stripe[:, 0:128].rearrange("(p k) f -> p k f", k=NTB)
                for k0, k1 in ((0, NTB // 2), (NTB // 2, NTB)):
                    nc.sync.dma_start(tpm[:, k0:k1, :], staging[:, k0:k1, :])
                if not DBG_NO_COLL:
                    nc.gpsimd.collective_compute(
                        "AllGather", mybir.AluOpType.bypass,
                        replica_groups=[list(range(NC))],
                        ins=[stripe.opt()], outs=[hws_full.opt()])

                # gathers issued lazily in consumption order; aggregate
                # via DoubleRow fp8 matmuls; relu epilogue with exact
                # dis[dst] as the ACT per-partition scale. In the last layer
                # the trailing chunks are split finer so the final blocks'
                # readout chains aren't backlogged behind one wide transfer.
                lo_chunks, hi_chunks = {}, {}

                def mk_plan(slots):
                    starts, s = [], 0
                    while s < slots:
                        starts.append(s)
                        s += min(GCH, slots - s)
                    return starts

                lo_plan = mk_plan(LO_SLOTS)
                hi_plan = mk_plan(HI_SLOTS)

                def col_chunk(plan, col):
                    # chunk id + col offset for tile-column `col`
                    ci = bisect.bisect_right(plan, col * 128) - 1
                    return ci, col - plan[ci] // 128

                def get_chunk(done, ci, plan, slots, idx, base_hi, tg):
                    if ci not in done:
                        s0 = plan[ci]
                        n = (plan[ci + 1] if ci + 1 < len(plan)
                             else slots) - s0
                        m = mp.tile([128, GCH // 128, 128], fp8, tag=tg,
                                    bufs=4)
                        if DBG_NO_GATHER:
                            nc.vector.memset(m[:], 0.0)
                        else:
                            src_ap = (hws_full[HI_BASE:, 0:128] if base_hi
                                      else hws_full[:, 0:128])
                            _dma_gather_128(
                                nc, m[:, : n // 128, :], src_ap,
                                idx[:, s0 // 16:(s0 + n) // 16], n)
                        done[ci] = m
                    return done[ci]

                # prefetch: hi chunks have no dep on this core's table write
                # in the measured program, so they can fill the DMA engines
                # across the table-write + first-lo-descgen window. Exactly
                # ring-depth many go ahead of lo chunk 0 (one more would
                # WAR-wait on hi chunk 0's consumers, which need lo chunk 0
                # -> deadlock on the in-order Pool queue).
                for ci in range(min(4, len(hi_plan))):
                    get_chunk(hi_chunks, ci, hi_plan, HI_SLOTS, idx_hi,
                              True, "mhi")
                if layer == 0:
                    # the s_part head piece rides the Pool queue between the
                    # hi prefetches and lo chunk 0, pinned behind the table
                    # write by a 128B table read whose output the load then
                    # overwrites (WAW keeps the scheduler from hoisting it):
                    # its transfer covers lo chunk 0's desc-gen window
                    nc.gpsimd.dma_start(s_sb[0:1, SG0:SG0 + 128],
                                        hws_full[0:1, 0:128])
                    nc.gpsimd.dma_start(s_sb[:, SG0:SGM],
                                        din["s_part"][:, 0:SGM - SG0])
                get_chunk(lo_chunks, 0, lo_plan, LO_SLOTS, idx_lo,
                          False, "mlo")

                if last and not DBG_NO_READOUT:
                    # reuse the idle "wmm" ring (no W matmuls in last layer)
                    pall_mean = ps_w.tile([128, D], f32, tag="wmm")
                    pall_att = ps_w.tile([128, D], f32, tag="wmm")
                    pall_den = ps_p.tile([128, 1], f32, tag="pden")
                for b in range(NBLK):
                    acc = ps_agg.tile([128, D], f32, tag="agg")
                    # self-loop term: staging row v already holds
                    # fp8(dis[v]*(h@W)[v]); identity matmul adds it to acc,
                    # the relu epilogue's dis scale makes it dis^2*(h@W).
                    nc.tensor.matmul(
                        out=acc[:], lhsT=ident8[:], rhs=staging[:, b, :],
                        start=True, stop=False)
                    # conv bias: diag(1/(16 dis)) @ (16 conv_b replicated)
                    # -> inv_dis*conv_b, the epilogue dis scale -> conv_b
                    nc.tensor.matmul(
                        out=acc[:], lhsT=diag8[:, b, :],
                        rhs=convb_pre[:, layer * D:(layer + 1) * D],
                        start=False, stop=False)
                    # plan matmuls: DoubleRow pairs where chunk-aligned,
                    # plain fp8 matmuls for odd tails / chunk straddles
                    ops = []
                    for T, base_t, st, plan in ((LO_T, 0, 0, lo_plan),
                                                (HI_T, LO_T, 1, hi_plan)):
                        t = 0
                        while t < T:
                            col = b * T + t
                            if (t + 1 < T and col_chunk(plan, col)[0]
                                    == col_chunk(plan, col + 1)[0]):
                                ops.append((st, T, base_t, t, 2))
                                t += 2
                            else:
                                ops.append((st, T, base_t, t, 1))
                                t += 1
                    for k, (st, T, base_t, t, w) in enumerate(ops):
                        col = b * T + t
                        if st == 0:
                            ci, cc = col_chunk(lo_plan, col)
                            mm = get_chunk(lo_chunks, ci, lo_plan, LO_SLOTS,
                                           idx_lo, False, "mlo")
                        else:
                            ci, cc = col_chunk(hi_plan, col)
                            mm = get_chunk(hi_chunks, ci, hi_plan, HI_SLOTS,
                                           idx_hi, True, "mhi")
                        sc0 = (b * NT + base_t + t) * 128
                        if w == 2:
                            nc.tensor.matmul(
                                out=acc[:],
                                lhsT=s_sb[:, sc0:sc0 + 256].rearrange(
                                    "p (i d) -> p i d", i=2),
                                rhs=mm[:, cc:cc + 2, :],
                                start=False, stop=(k == len(ops) - 1),
                                perf_mode=mybir.MatmulPerfMode.DoubleRow)
                        else:
                            nc.tensor.matmul(
                                out=acc[:],
                                lhsT=s_sb[:, sc0:sc0 + 128],
                                rhs=mm[:, cc, :],
                                start=False, stop=(k == len(ops) - 1))
                    nc.scalar.activation(
                        h_nm[:, b, :], acc[:], AF.Relu,
                        scale=dis_nm[:, b:b + 1])
                    if layer == 0 and b + SGEN_AHEAD < SGEN_BLOCKS:
                        emit_sgen(b + SGEN_AHEAD)
                    if not last or DBG_DUMP_H:
                        pt = ps_tr.tile([128, 128], bft, tag="ptr")
                        nc.tensor.transpose(
                            out=pt[:], in_=h_nm[:, b, :],
                            identity=tident[:])
                        # PSUM->SBUF copy: on Act while layer 0's DVE is
                        # saturated by the S generation, on DVE otherwise
                        if layer == 0 and b < SGEN_BLOCKS:
                            nc.scalar.activation(
                                h_fm[:, b * 128:(b + 1) * 128], pt[:],
                                AF.Copy)
                        else:
                            nc.vector.tensor_copy(
                                h_fm[:, b * 128:(b + 1) * 128], pt[:])
                    if not last:
                        # next layer's table chunk, pipelined under this
                        # layer's gather phase
                        emit_wcast(layer + 1, b)
                    elif not DBG_NO_READOUT:
                        # readout pieces that only need h_nm[b]: scores
                        # (DVE mul+reduce), the mean-pool matmul, and the
                        # attention-pool matmul (softmax denominator is
                        # folded in at the end, so exp/mask/pool are all
                        # per-block; scores here are O(0.1) so exp() is
                        # overflow-safe without the usual max shift)
                        tmp = rp2.tile([128, D], bft, tag="sc")
                        nc.vector.tensor_mul(tmp[:], h_nm[:, b, :],
                                             attrep[:])
                        nc.vector.tensor_reduce(
                            out=sc_nm[:, b:b + 1], in_=tmp[:],
                            axis=mybir.AxisListType.X,
                            op=mybir.AluOpType.add)
                        nc.tensor.matmul(
                            out=pall_mean[0:PPC, :],
                            lhsT=pone_cj[:, b * PPC:(b + 1) * PPC],
                            rhs=h_nm[:, b, :],
                            start=(b == 0), stop=(b == NBLK - 1),
                            skip_group_check=True)
                        nc.scalar.activation(exl[:, b:b + 1],
                                             sc_nm[:, b:b + 1], AF.Exp)
                        nc.vector.tensor_mul(exl[:, b:b + 1],
                                             exl[:, b:b + 1],
                                             lys_nm[:, b:b + 1])
                        nc.vector.tensor_scalar_mul(
                            pex_all[:, b, :],
                            pone[:, b * PPC:(b + 1) * PPC],
                            exl[:, b:b + 1])
                        nc.tensor.matmul(
                            out=pall_att[0:PPC, :],
                            lhsT=pex_all[:, b, :], rhs=h_nm[:, b, :],
                            start=(b == 0), stop=(b == NBLK - 1),
                            skip_group_check=True)
                        nc.tensor.matmul(
                            out=pall_den[0:PPC, :],
                            lhsT=pex_all[:, b, :], rhs=ones_bf[:],
                            start=(b == 0), stop=(b == NBLK - 1),
                            skip_group_check=True)

            if DBG_DUMP_H:
                for b in range(NTB):
                    nc.gpsimd.dma_start(
                        out_h[:, b * 128:(b + 1) * 128],
                        h_fm[:, b * 128:(b + 1) * 128])
            if DBG_DUMP_TB:
                nc.gpsimd.dma_start(
                    out_tb[:].rearrange("p (k f) -> p k f", k=NTB),
                    staging[:])

            if DBG_NO_READOUT:
                oz = rp2.tile([64, PPC], f32, tag="oz")
                nc.vector.tensor_copy(oz[:], h_nm[0:64, 0, 0:PPC])
                nc.gpsimd.dma_start(out_t[:], oz[:])

            if not DBG_NO_READOUT:
                # c_j = 1/(max(cnt,1)*sqrt(cnt+1e-6)) is folded into pone_cj
                # host-side, so the protein term comes out of its matmul
                # pre-scaled and the combine is one fused DVE op
                dg = gp.tile([PPC, 1], f32)
                nc.vector.tensor_scalar_max(dg[:], pall_den[0:PPC, :],
                                            1.0e-30)
                rden = gp.tile([PPC, 1], f32)
                nc.vector.reciprocal(rden[:], dg[:])

                lw = gp.tile([PPC, 128], f32)
                nc.vector.tensor_scalar_mul(lw[:], pall_att[0:PPC, :],
                                            rden[:])
                pre = gp.tile([PPC, 128], f32)
                nc.vector.tensor_add(pre[:], lw[:], pall_mean[0:PPC, :])
                ptp = ps_r.tile([128, 128], f32, tag="tr")
                nc.tensor.transpose(
                    out=ptp[:, 0:PPC], in_=pre[:],
                    identity=ident[0:PPC, 0:PPC])
                preT = gp.tile([128, PPC], f32)
                nc.vector.tensor_copy(preT[:], ptp[:, 0:PPC])
                pso = ps_r.tile([128, 128], f32, tag="tr")
                nc.tensor.matmul(
                    out=pso[0:64, 0:PPC], lhsT=outw[:], rhs=preT[:],
                    start=True, stop=True)
                osb = gp.tile([64, PPC], f32)
                nc.vector.tensor_scalar_add(osb[:], pso[0:64, 0:PPC],
                                            outb[:])
                nc.sync.dma_start(out_t[:], osb[:])

    nc.compile()
    return nc


# ---------------------------------------------------------------- entry

def kernel(**inputs):
    x = np.asarray(inputs["x"], np.float32)
    edge_index = np.asarray(inputs["edge_index"])
    batch = np.asarray(inputs["batch"])
    lysine_mask = np.asarray(inputs["lysine_mask"])
    conv_w = np.asarray(inputs["conv_w"], np.float32)
    conv_b = np.asarray(inputs["conv_b"], np.float32)
    att_w = np.asarray(inputs["att_w"], np.float32)
    out_w = np.asarray(inputs["out_w"], np.float32)
    out_b = np.asarray(inputs["out_b"], np.float32)

    per_core, LO_T, HI_T, NT = _host_prep(x, edge_index, batch, lysine_mask)

    convw = np.ascontiguousarray(
        np.concatenate([conv_w[i] for i in range(L)], axis=1)).astype(bf16)
    convb_pre = np.tile(
        np.concatenate([16.0 * conv_b[i] for i in range(L)]).astype(E4),
        (128, 1))
    shared = dict(
        convw=convw, convb_pre=convb_pre,
        attw_row=att_w.reshape(1, D).astype(np.float32),
        outw=out_w.astype(np.float32),
        outb=out_b.reshape(64, 1).astype(np.float32),
    )
    in_maps = []
    for c in range(NC):
        pc = per_core[c]
        in_maps.append({
            "x_t": pc["x_t"], "rowdst": pc["rowdst"],
            "iota_row": pc["iota_row"], "inv16": pc["inv16"],
            "s_part": pc["s_part"],
            "idx_lo": pc["idx_lo"], "idx_hi": pc["idx_hi"],
            "dis_nm": pc["dis_nm"],
            "pone": pc["pone"], "pone_cj": pc["pone_cj"],
            "lys_nm": pc["lys_nm"], **shared,
        })

    nc_prog = _build_program(LO_T, HI_T, NT)
    trace = os.environ.get("GCN_TRACE", "") == "1"
    res = run_bass_kernel_spmd(
        nc_prog, in_maps, core_ids=list(range(NC)), trace=trace)
    if trace:
        import kernel as _self
        _self.LAST_RESULT = res
        print("HW exec time:", res.exec_time_ns, "ns")
    out = np.concatenate(
        [np.asarray(res.results[c]["out_t"], np.float32).T for c in range(NC)],
        axis=0)
    return out



# revision 101
# speedup vs baseline: 1.0218x; 1.0218x over previous
"""GCN message-passing kernel for Trainium2, 8 NeuronCores (SPMD).

Strategy (graph-parallel, fp8 messages):
- Nodes are protein-contiguous, sharded across 8 cores at protein boundaries
  (16 proteins/core, padded to 6400 nodes/core). Within a core, nodes are
  bin-packed into 50 blocks of 128 balancing incoming-edge counts; slot
  s = blk*128 + pos maps to partition pos, chunk blk everywhere (h, dis,
  table, S, pooling) so aggregation blocks coincide with node chunks.
- Message table is fp8e4m3 at 256B row stride with a 128B payload
  (row = dis[src] * (h @ W)[src]); gathers use elem_size=128/elem_step=256
  (bass's %256 payload assert is bypassed via direct InstDMAGatherAnt
  construction - verified byte-exact on hardware), halving per-edge DMA
  cost vs bf16. Only real edges are gathered: the self-loop term is an
  identity matmul on the resident staging chunk, and the conv bias is a
  per-block diag(1/(16 dis)) @ (16 conv_b) matmul - both accumulate into
  the same PSUM group, so the gather stream is edges-only (NT=12 tiles
  per block instead of 13).
- Aggregation is node-major: acc[dst,feat] = S^T @ msgs with S a 0/1
  one-hot (exact in fp8) via DoubleRow fp8 matmuls (256 slots each);
  dis[dst] is applied EXACTLY by the relu epilogue's per-partition ACT
  scale. S rows are pure one-hots, so the first SGEN_BLOCKS blocks of S
  are generated on-chip (one DVE is_equal per block against an iota row,
  from a compact per-slot dst-position table) and only the rest load from
  DRAM - the DMA engines are the global bottleneck, DVE has slack.
- Pipeline: hi-stream gather chunks carry no dependency on the measured
  program's table write, so ring-depth many are prefetched to cover each
  layer's table-write + first-lo-descgen window; the table is written in
  halves so early wcasts stream out under the previous layer's gathers.
- h is kept bf16 both node-major (epilogue output; feeds readout) and
  feature-major (one PE transpose per chunk per layer; copies on Act
  while DVE generates S, on DVE afterwards).
- Readout is per-block in the last layer: scores via DVE mul+reduce
  against a broadcast att_w row, exp/mask immediately (denominator folded
  in at the end - softmax shift is skipped since scores are O(0.1)), and
  three accumulating pool matmuls (mean with host-folded 1/(n sqrt n),
  attention numerator, denominator); the final combine, transpose and
  projection run once per core after the block loop.
"""
import bisect
import os
import numpy as np
import ml_dtypes

DBG_LAYERS = int(os.environ.get("GCN_DBG_LAYERS", "4"))
DBG_NO_COLL = os.environ.get("GCN_DBG_NO_COLL", "") == "1"
DBG_NO_GATHER = os.environ.get("GCN_DBG_NO_GATHER", "") == "1"
DBG_NO_READOUT = os.environ.get("GCN_DBG_NO_READOUT", "") == "1"
DBG_DUMP_H = os.environ.get("GCN_DBG_DUMP_H", "") == "1"
DBG_DUMP_TB = os.environ.get("GCN_DBG_DUMP_TB", "") == "1"

import concourse.bacc as bacc
import concourse.tile as tile
import concourse.tile_utils as tile_utils
from concourse import mybir
from concourse.bass_utils import run_bass_kernel_spmd
from concourse.masks import make_identity

bf16 = ml_dtypes.bfloat16
E4 = ml_dtypes.float8_e4m3
AF = mybir.ActivationFunctionType

NC = 8
D = 128
L = 4
B = 128
PPC = B // NC          # proteins per core
NPAD = 6400            # padded nodes per core
NPADG = NC * NPAD      # global padded rows
NTB = NPAD // 128      # 50 chunks of 128 nodes == aggregation blocks
NBLK = NTB
LO_BOUND = 32000       # lo gather covers rows [0, 32000)
HI_BASE = 18560        # hi gather covers rows [18560, 51200): 32639 <= int16
GCH = 8192             # gather slots per dma_gather instruction (64 cols)
SGEN_BLOCKS = 28       # S blocks generated on-chip; the rest load from DRAM

f32 = mybir.dt.float32
bft = mybir.dt.bfloat16
fp8 = mybir.dt.float8e4
i16 = mybir.dt.int16


# ---------------------------------------------------------------- host prep

def _pack_idx(vals, slots):
    """int16 gather index layout: position i -> partition i%16, col i//16,
    replicated across the 128 partitions."""
    assert len(vals) == slots and slots % 16 == 0
    arr = np.asarray(vals, np.int16).reshape(slots // 16, 16).T  # [16, s//16]
    return np.ascontiguousarray(np.tile(arr, (8, 1)))


def _ceil128(x):
    return max(1, int(np.ceil(x / 128)))


def _host_prep(x, edge_index, batch, lysine_mask):
    N = x.shape[0]
    src = np.asarray(edge_index[0], np.int64)
    dst = np.asarray(edge_index[1], np.int64)
    batch = np.asarray(batch, np.int64)

    pcounts = np.bincount(batch, minlength=B)
    pstart = np.concatenate([[0], np.cumsum(pcounts)])
    cstart = pstart[np.arange(NC) * PPC]
    cend = pstart[(np.arange(NC) + 1) * PPC]
    ncore = cend - cstart
    assert ncore.max() <= NPAD - 1, f"core node count {ncore.max()} > {NPAD-1}"
    assert pcounts.max() <= 128 * NTB

    deg = np.bincount(dst, minlength=N).astype(np.float64) + 1.0
    dis = (1.0 / np.sqrt(deg)).astype(np.float32)
    core_of = np.searchsorted(cend, np.arange(N), side="right")

    # --- per-core node packing into NBLK blocks of 128, balancing in-slot
    # (in-edges + self) counts per block; (pos 127, blk 49) is reserved.
    blk = np.zeros(N, np.int64)
    pos = np.zeros(N, np.int64)
    for c in range(NC):
        nodes = np.arange(cstart[c], cend[c])
        tot = deg[nodes]
        order = np.argsort(-tot, kind="stable")
        caps = np.full(NBLK, 128, np.int64)
        caps[NBLK - 1] = 127
        loads = np.zeros(NBLK)
        cnts = np.zeros(NBLK, np.int64)
        for i in order:
            masked = np.where(cnts < caps, loads, np.inf)
            b = int(np.argmin(masked))
            blk[nodes[i]] = b
            # (pos 0, blk 49) is the reserved bias slot on every core
            pos[nodes[i]] = cnts[b] + (1 if b == NBLK - 1 else 0)
            cnts[b] += 1
            loads[b] += tot[i]
    slot = blk * 128 + pos                    # local pi slot
    grow = core_of * NPAD + pos * NTB + blk   # global table row

    # --- edge list: real edges only. Self-loops are applied on-chip via an
    # identity matmul on the staging chunk; the conv bias enters via a
    # per-block diagonal matmul (diag(1/(16 dis)) @ 16*conv_b), so there are
    # no pseudo-edges and every S row is a pure one-hot — which lets S be
    # GENERATED on-chip (DVE is_equal against an iota row) from a compact
    # per-slot dst-position table (rowdst) instead of a 10MB fp8 load.
    e_row = grow[src]
    e_core = core_of[dst]
    e_blk = blk[dst]
    e_col = pos[dst]

    cls = np.where(e_row < HI_BASE, 0,
                   np.where(e_row < LO_BOUND, 1, 2))
    key = e_core * NBLK + e_blk
    nl0 = np.bincount(key[cls == 0], minlength=NC * NBLK)
    nf = np.bincount(key[cls == 1], minlength=NC * NBLK)
    tot_cb = np.bincount(key, minlength=NC * NBLK)

    best = None
    for LO_T in range(_ceil128(nl0.max()), _ceil128(nl0.max()) + 4):
        lo_fill = np.minimum(LO_T * 128, nl0 + nf)
        HI_T = _ceil128((tot_cb - lo_fill).max())
        if best is None or LO_T + HI_T < best[0] + best[1]:
            best = (LO_T, HI_T)
    LO_T, HI_T = best
    NT = LO_T + HI_T

    iota_row = np.tile(np.arange(128, dtype=np.float32), (128, 1))
    per_core = []
    for c in range(NC):
        m = e_core == c
        rows_e, blk_e, col_e, cls_e = (
            e_row[m], e_blk[m], e_col[m], cls[m])
        order = np.lexsort((col_e, cls_e, blk_e))
        rows_e, blk_e, col_e, cls_e = (
            rows_e[order], blk_e[order], col_e[order], cls_e[order])
        bstart = np.searchsorted(blk_e, np.arange(NBLK))
        bend = np.searchsorted(blk_e, np.arange(NBLK), side="right")

        nodes = np.arange(cstart[c], cend[c])
        # dis / inv-dis in pi layout (pads -> 1 / 0)
        dis_nm = np.ones((128, NTB), np.float32)
        inv_nm = np.zeros((128, NTB), np.float32)
        dis_nm[pos[nodes], blk[nodes]] = dis[nodes]
        inv_nm[pos[nodes], blk[nodes]] = 1.0 / dis[nodes]

        lo_idx = np.zeros(NBLK * LO_T * 128, np.int64)
        hi_idx = np.zeros(NBLK * HI_T * 128, np.int64)  # already HI_BASE-offset
        # per-slot dst position, 255 for padding (never matches iota 0..127)
        rowdst = np.full((128, NBLK * NT), 255.0, np.float32)
        s_all = np.zeros((128, NBLK * NT * 128), np.float32)
        for b in range(NBLK):
            sl = slice(bstart[b], bend[b])
            r_b, c_b, k_b = rows_e[sl], col_e[sl], cls_e[sl]
            n = len(r_b)
            n0 = int((k_b == 0).sum())
            nfb = int((k_b == 1).sum())
            take = min(LO_T * 128 - n0, nfb)
            assert take >= 0, f"block lo overflow {n0} > {LO_T*128}"
            nlo = n0 + take
            nhi = n - nlo
            assert nhi <= HI_T * 128
            for stream, cnt, off, idxarr, base_t, ibase in (
                (0, nlo, 0, lo_idx, 0, 0),
                (1, nhi, nlo, hi_idx, LO_T, HI_BASE),
            ):
                if cnt == 0:
                    continue
                rr = r_b[off:off + cnt] - ibase
                cc = c_b[off:off + cnt]
                T = LO_T if stream == 0 else HI_T
                idxarr[b * T * 128: b * T * 128 + cnt] = rr
                k = np.arange(cnt)
                p = k % 128
                t = base_t + k // 128
                rowdst[p, b * NT + t] = cc
                s_all[p, (b * NT + t) * 128 + cc] = 1.0

        x_t = np.zeros((D, NPAD), np.float32)
        x_t[:, slot[nodes]] = np.asarray(x[nodes], np.float32).T

        lens = pcounts[c * PPC:(c + 1) * PPC]
        starts = np.concatenate([[0], np.cumsum(lens)])[:-1]
        q = np.arange(ncore[c])
        pj = np.searchsorted(starts, q, side="right") - 1
        pone = np.zeros((128, NTB * PPC), bf16)
        pone[pos[nodes], blk[nodes] * PPC + pj] = 1.0
        lens_f = np.asarray(lens, np.float64)
        cj = (1.0 / (np.maximum(lens_f, 1.0) * np.sqrt(lens_f + 1e-6)))
        pone_cj = (pone.astype(np.float32)
                   * np.tile(cj, NTB)[None, :]).astype(bf16)
        lys_nm = np.zeros((128, NTB), np.float32)
        lys_nm[pos[nodes], blk[nodes]] = np.asarray(
            lysine_mask[nodes], np.float32)

        per_core.append(dict(
            x_t=x_t.astype(bf16),
            rowdst=rowdst,
            iota_row=iota_row,
            inv16=inv_nm / 16.0,
            s_part=np.ascontiguousarray(
                s_all[:, SGEN_BLOCKS * NT * 128:]).astype(E4),
            idx_lo=_pack_idx(lo_idx, NBLK * LO_T * 128),
            idx_hi=_pack_idx(hi_idx, NBLK * HI_T * 128),
            dis_nm=dis_nm,
            pone=pone,
            pone_cj=pone_cj,
            lys_nm=lys_nm,
        ))
    return per_core, LO_T, HI_T, NT


# ---------------------------------------------------------------- program

def _dma_gather_128(nc, out_ap, in_ap, idxs_ap, num_idxs):
    """dma_gather with a 128B payload on a 256B-stride table (elem_size=128
    fp8, elem_step=256). Bypasses bass's %256 payload assert; verified
    byte-exact on hardware."""
    g = nc.gpsimd
    _in_ap = g.lower_ap_dma(in_ap, for_custom_bir_dma=True)
    _idxs_ap = g.lower_ap(idxs_ap)
    _out_ap = g.lower_ap(out_ap)
    return g.add_instruction(mybir.InstDMAGatherAnt(
        name=g.bass.get_next_instruction_name(),
        ins=[*_in_ap, _idxs_ap, g.lower_val_access(g.to_reg(num_idxs))],
        outs=[_out_ap],
        transpose=False, num_idxs=num_idxs, elem_size=128,
        stride_bytes_256=1, gen_mode=0, single_packet=False,
        queue_num=0, sbuf_tokens_per_rank=0, sbuf_free_dim_per_rank=0,
        sbuf_free_dim_pad_per_rank=0, sbuf_byte_offset=0))


def _build_program(LO_T, HI_T, NT):
    tile_utils.max_sbuf_usage = 208 * 1024
    nc = bacc.Bacc("TRN2", target_bir_lowering=False, num_devices=NC,
                   num_swdge_queues=2)

    din = {}
    for name, shape, dt in [
        ("x_t", [D, NPAD], bft),
        ("rowdst", [128, NBLK * NT], f32),
        ("iota_row", [128, 128], f32),
        ("inv16", [128, NTB], f32),
        ("s_part", [128, (NBLK - SGEN_BLOCKS) * NT * 128], fp8),
        ("idx_lo", [128, NBLK * LO_T * 8], i16),
        ("idx_hi", [128, NBLK * HI_T * 8], i16),
        ("dis_nm", [128, NTB], f32),
        ("pone", [128, NTB * PPC], bft),
        ("pone_cj", [128, NTB * PPC], bft),
        ("lys_nm", [128, NTB], f32),
        ("convw", [D, L * D], bft),
        ("convb_pre", [128, L * D], fp8),
        ("attw_row", [1, D], f32),
        ("outw", [D, 64], f32),
        ("outb", [64, 1], f32),
    ]:
        din[name] = nc.dram_tensor(name, shape, dt, kind="ExternalInput")
    out_t = nc.dram_tensor("out_t", [64, PPC], f32, kind="ExternalOutput")
    out_h = None
    if DBG_DUMP_H:
        out_h = nc.dram_tensor("out_h", [128, NPAD], bft,
                               kind="ExternalOutput")
    out_tb = None
    if DBG_DUMP_TB:
        out_tb = nc.dram_tensor("out_tb", [128, NTB * 128], fp8,
                                kind="ExternalOutput")

    LO_SLOTS = NBLK * LO_T * 128
    HI_SLOTS = NBLK * HI_T * 128

    with tile.TileContext(nc) as tc:
        with (
            tc.tile_pool(name="glob", bufs=1) as gp,
            tc.tile_pool(name="dram", bufs=1, space="DRAM") as dram,
            tc.tile_pool(name="msgs", bufs=4) as mp,
            tc.tile_pool(name="r2", bufs=2) as rp2,
            tc.tile_pool(name="ps_w", bufs=2, space="PSUM") as ps_w,
            tc.tile_pool(name="ps_agg", bufs=3, space="PSUM") as ps_agg,
            tc.tile_pool(name="ps_tr", bufs=1, space="PSUM") as ps_tr,
            tc.tile_pool(name="ps_r", bufs=1, space="PSUM") as ps_r,
            tc.tile_pool(name="ps_p", bufs=1, space="PSUM") as ps_p,
        ):
            # resident SBUF state
            h_fm = gp.tile([D, NPAD], bft, name="h_fm")
            nc.sync.dma_start(h_fm[:], din["x_t"][:])
            h_nm = gp.tile([128, NTB, 128], bft, name="h_nm")
            staging = gp.tile([128, NTB, 128], fp8, name="staging")
            s_sb = gp.tile([128, NBLK * NT * 128], fp8, name="s_sb")
            # idx_hi loads first (it gates the first hi-gather desc-gen),
            # then the wcast deps (convw/dis/convb), then the rest
            idx_hi = gp.tile([128, HI_SLOTS // 16], i16)
            nc.sync.dma_start(idx_hi[:], din["idx_hi"][:])
            convw = gp.tile([D, L * D], bft)
            nc.sync.dma_start(convw[:], din["convw"][:])
            dis_nm = gp.tile([128, NTB], f32)
            nc.sync.dma_start(dis_nm[:], din["dis_nm"][:])
            convb_pre = gp.tile([128, L * D], fp8)
            nc.sync.dma_start(convb_pre[:], din["convb_pre"][:])
            idx_lo = gp.tile([128, LO_SLOTS // 16], i16)
            nc.sync.dma_start(idx_lo[:], din["idx_lo"][:])
            rowdst = gp.tile([128, NBLK * NT], f32)
            nc.sync.dma_start(rowdst[:], din["rowdst"][:])
            iota_row = gp.tile([128, 128], f32)
            nc.sync.dma_start(iota_row[:], din["iota_row"][:])
            inv16 = gp.tile([128, NTB], f32)
            nc.sync.dma_start(inv16[:], din["inv16"][:])
            pone = gp.tile([128, NTB * PPC], bft)
            nc.sync.dma_start(pone[:], din["pone"][:])
            lys_nm = gp.tile([128, NTB], f32)
            nc.sync.dma_start(lys_nm[:], din["lys_nm"][:])
            pone_cj = gp.tile([128, NTB * PPC], bft)
            nc.sync.dma_start(pone_cj[:], din["pone_cj"][:])
            attw = gp.tile([1, D], f32)
            nc.sync.dma_start(attw[:], din["attw_row"][:])
            outw = gp.tile([D, 64], f32)
            nc.sync.dma_start(outw[:], din["outw"][:])
            outb = gp.tile([64, 1], f32)
            nc.sync.dma_start(outb[:], din["outb"][:])
            stripe = dram.tile([NPAD, 256], fp8)
            hws_full = dram.tile([NPADG, 256], fp8)
            tident = gp.tile([128, 128], bft)
            make_identity(nc, tident[:])
            ident = gp.tile([128, 128], f32)
            make_identity(nc, ident[:])
            ident8 = gp.tile([128, 128], fp8)
            make_identity(nc, ident8[:])
            ones_r = gp.tile([1, 128], f32)
            nc.vector.memset(ones_r[:], 1.0)
            ones_f = gp.tile([128, 1], f32)
            nc.vector.memset(ones_f[:], 1.0)
            ones_bf = gp.tile([128, 1], bft)
            nc.vector.memset(ones_bf[:], 1.0)

            # att_w broadcast to all partitions (ones outer product)
            psat = ps_r.tile([128, D], f32, tag="tr")
            nc.tensor.matmul(out=psat[:], lhsT=ones_r[:],
                             rhs=attw[:], start=True, stop=True)
            attrep = gp.tile([128, D], bft)
            nc.vector.tensor_copy(attrep[:], psat[:])
            sc_nm = gp.tile([128, NTB], f32)
            exl = gp.tile([128, NTB], f32)
            pex_all = gp.tile([128, NTB, PPC], bft)

            # per-block diag(1/(16 dis[dst])) in fp8: the bias matmul's lhsT
            # (same quantization as the old in-S bias column); filled
            # per-block alongside the S generation / load below
            diag8 = gp.tile([128, NTB, 128], fp8)

            def emit_sgen(b):
                # S block b: one-hot rows from per-slot dst positions via a
                # single DVE is_equal (iota row tiled along tiles; rowdst
                # broadcast along the 128 dst columns). Padding slots carry
                # 255 and generate all-zero rows. ~1.66us per block on DVE,
                # which only keeps ahead of the layer-0 aggregation pace for
                # the first SGEN_BLOCKS blocks — the rest load from DRAM.
                nc.vector.tensor_tensor(
                    out=s_sb[:, b * NT * 128:(b + 1) * NT * 128].rearrange(
                        "p (t c) -> p t c", t=NT),
                    in0=iota_row[:].rearrange(
                        "p (o c) -> p o c", o=1).broadcast_to([128, NT, 128]),
                    in1=rowdst[:, b * NT:(b + 1) * NT].rearrange(
                        "p (t o) -> p t o", o=1).broadcast_to([128, NT, 128]),
                    op=mybir.AluOpType.is_equal)
                nc.vector.tensor_scalar_mul(
                    diag8[:, b, :], ident8[:], inv16[:, b:b + 1])

            def emit_wcast(layer, b):
                # table chunk: staging[:, b, :] = fp8(dis * (h @ W)),
                # node-major via out = h_fm_chunk^T @ W
                pw = ps_w.tile([128, D], f32, tag="wmm")
                nc.tensor.matmul(
                    out=pw[:],
                    lhsT=h_fm[:, b * 128:(b + 1) * 128],
                    rhs=convw[:, layer * D:(layer + 1) * D],
                    start=True, stop=True)
                nc.scalar.activation(
                    staging[:, b, :], pw[:], AF.Copy,
                    scale=dis_nm[:, b:b + 1])

            SGEN_AHEAD = 10
            for b in range(NTB):
                emit_wcast(0, b)
            for b in range(min(SGEN_AHEAD, SGEN_BLOCKS)):
                emit_sgen(b)
            for b in range(SGEN_BLOCKS, NBLK):
                nc.vector.tensor_scalar_mul(
                    diag8[:, b, :], ident8[:], inv16[:, b:b + 1])
            # S for the later half of the loaded blocks arrives early on the
            # Act queue; the first piece is issued behind layer 0's table
            # write on the SP queue, where it fills the DMA engines during
            # the first lo gather's desc-gen window
            SG0 = SGEN_BLOCKS * NT * 128
            SGM = ((SGEN_BLOCKS + NBLK) // 2) * NT * 128
            nc.scalar.dma_start(s_sb[:, SGM:],
                                din["s_part"][:, SGM - SG0:])

            pall_mean = None
            pall_att = None
            for layer in range(DBG_LAYERS):
                last = layer == DBG_LAYERS - 1
                # table write in pieces so early-chunk payloads stream out
                # while later wcasts still run — the next layer's lo gathers
                # wait on all of them
                if DBG_NO_COLL:
                    # collective stand-in: write the staged payload straight
                    # into this core's own region of the table (same local
                    # DMA work as the real path's stripe write)
                    tpm = hws_full[0:NPAD, 0:128].rearrange(
                        "(p k) f -> p k f", k=NTB)
                else:
                    tpm = stripe[:, 0:128].rearrange("(p k) f -> p k f", k=NTB)
                for k0, k1 in ((0, NTB // 2), (NTB // 2, NTB)):
                    nc.sync.dma_start(tpm[:, k0:k1, :], staging[:, k0:k1, :])
                if not DBG_NO_COLL:
                    nc.gpsimd.collective_compute(
                        "AllGather", mybir.AluOpType.bypass,
                        replica_groups=[list(range(NC))],
                        ins=[stripe.opt()], outs=[hws_full.opt()])

                # gathers issued lazily in consumption order; aggregate
                # via DoubleRow fp8 matmuls; relu epilogue with exact
                # dis[dst] as the ACT per-partition scale. In the last layer
                # the trailing chunks are split finer so the final blocks'
                # readout chains aren't backlogged behind one wide transfer.
                lo_chunks, hi_chunks = {}, {}

                def mk_plan(slots):
                    starts, s = [], 0
                    while s < slots:
                        starts.append(s)
                        s += min(GCH, slots - s)
                    return starts

                lo_plan = mk_plan(LO_SLOTS)
                hi_plan = mk_plan(HI_SLOTS)

                def col_chunk(plan, col):
                    # chunk id + col offset for tile-column `col`
                    ci = bisect.bisect_right(plan, col * 128) - 1
                    return ci, col - plan[ci] // 128

                def get_chunk(done, ci, plan, slots, idx, base_hi, tg):
                    if ci not in done:
                        s0 = plan[ci]
                        n = (plan[ci + 1] if ci + 1 < len(plan)
                             else slots) - s0
                        m = mp.tile([128, GCH // 128, 128], fp8, tag=tg,
                                    bufs=4)
                        if DBG_NO_GATHER:
                            nc.vector.memset(m[:], 0.0)
                        else:
                            src_ap = (hws_full[HI_BASE:, 0:128] if base_hi
                                      else hws_full[:, 0:128])
                            _dma_gather_128(
                                nc, m[:, : n // 128, :], src_ap,
                                idx[:, s0 // 16:(s0 + n) // 16], n)
                        done[ci] = m
                    return done[ci]

                # prefetch: hi chunks have no dep on this core's table write
                # in the measured program, so they can fill the DMA engines
                # across the table-write + first-lo-descgen window. Exactly
                # ring-depth many go ahead of lo chunk 0 (one more would
                # WAR-wait on hi chunk 0's consumers, which need lo chunk 0
                # -> deadlock on the in-order Pool queue).
                for ci in range(min(4, len(hi_plan))):
                    get_chunk(hi_chunks, ci, hi_plan, HI_SLOTS, idx_hi,
                              True, "mhi")
                if layer == 0:
                    # the s_part head piece rides the Pool queue between the
                    # hi prefetches and lo chunk 0, pinned behind the table
                    # write by a 128B table read whose output the load then
                    # overwrites (WAW keeps the scheduler from hoisting it):
                    # its transfer covers lo chunk 0's desc-gen window
                    nc.gpsimd.dma_start(s_sb[0:1, SG0:SG0 + 128],
                                        hws_full[0:1, 0:128])
                    nc.gpsimd.dma_start(s_sb[:, SG0:SGM],
                                        din["s_part"][:, 0:SGM - SG0])
                get_chunk(lo_chunks, 0, lo_plan, LO_SLOTS, idx_lo,
                          False, "mlo")

                if last and not DBG_NO_READOUT:
                    # reuse the idle "wmm" ring (no W matmuls in last layer)
                    pall_mean = ps_w.tile([128, D], f32, tag="wmm")
                    pall_att = ps_w.tile([128, D], f32, tag="wmm")
                    pall_den = ps_p.tile([128, 1], f32, tag="pden")
                for b in range(NBLK):
                    acc = ps_agg.tile([128, D], f32, tag="agg")
                    # self-loop term: staging row v already holds
                    # fp8(dis[v]*(h@W)[v]); identity matmul adds it to acc,
                    # the relu epilogue's dis scale makes it dis^2*(h@W).
                    nc.tensor.matmul(
                        out=acc[:], lhsT=ident8[:], rhs=staging[:, b, :],
                        start=True, stop=False)
                    # conv bias: diag(1/(16 dis)) @ (16 conv_b replicated)
                    # -> inv_dis*conv_b, the epilogue dis scale -> conv_b
                    nc.tensor.matmul(
                        out=acc[:], lhsT=diag8[:, b, :],
                        rhs=convb_pre[:, layer * D:(layer + 1) * D],
                        start=False, stop=False)
                    # plan matmuls: DoubleRow pairs where chunk-aligned,
                    # plain fp8 matmuls for odd tails / chunk straddles
                    ops = []
                    for T, base_t, st, plan in ((LO_T, 0, 0, lo_plan),
                                                (HI_T, LO_T, 1, hi_plan)):
                        t = 0
                        while t < T:
                            col = b * T + t
                            if (t + 1 < T and col_chunk(plan, col)[0]
                                    == col_chunk(plan, col + 1)[0]):
                                ops.append((st, T, base_t, t, 2))
                                t += 2
                            else:
                                ops.append((st, T, base_t, t, 1))
                                t += 1
                    for k, (st, T, base_t, t, w) in enumerate(ops):
                        col = b * T + t
                        if st == 0:
                            ci, cc = col_chunk(lo_plan, col)
                            mm = get_chunk(lo_chunks, ci, lo_plan, LO_SLOTS,
                                           idx_lo, False, "mlo")
                        else:
                            ci, cc = col_chunk(hi_plan, col)
                            mm = get_chunk(hi_chunks, ci, hi_plan, HI_SLOTS,
                                           idx_hi, True, "mhi")
                        sc0 = (b * NT + base_t + t) * 128
                        if w == 2:
                            nc.tensor.matmul(
                                out=acc[:],
                                lhsT=s_sb[:, sc0:sc0 + 256].rearrange(
                                    "p (i d) -> p i d", i=2),
                                rhs=mm[:, cc:cc + 2, :],
                                start=False, stop=(k == len(ops) - 1),
                                perf_mode=mybir.MatmulPerfMode.DoubleRow)
                        else:
                            nc.tensor.matmul(
                                out=acc[:],
                                lhsT=s_sb[:, sc0:sc0 + 128],
                                rhs=mm[:, cc, :],
                                start=False, stop=(k == len(ops) - 1))
                    nc.scalar.activation(
                        h_nm[:, b, :], acc[:], AF.Relu,
                        scale=dis_nm[:, b:b + 1])
                    if layer == 0 and b + SGEN_AHEAD < SGEN_BLOCKS:
                        emit_sgen(b + SGEN_AHEAD)
                    if not last or DBG_DUMP_H:
                        pt = ps_tr.tile([128, 128], bft, tag="ptr")
                        nc.tensor.transpose(
                            out=pt[:], in_=h_nm[:, b, :],
                            identity=tident[:])
                        # PSUM->SBUF copy: on Act while layer 0's DVE is
                        # saturated by the S generation, on DVE otherwise
                        if layer == 0 and b < SGEN_BLOCKS:
                            nc.scalar.activation(
                                h_fm[:, b * 128:(b + 1) * 128], pt[:],
                                AF.Copy)
                        else:
                            nc.vector.tensor_copy(
                                h_fm[:, b * 128:(b + 1) * 128], pt[:])
                    if not last:
                        # next layer's table chunk, pipelined under this
                        # layer's gather phase
                        emit_wcast(layer + 1, b)
                    elif not DBG_NO_READOUT:
                        # readout pieces that only need h_nm[b]: scores
                        # (DVE mul+reduce), the mean-pool matmul, and the
                        # attention-pool matmul (softmax denominator is
                        # folded in at the end, so exp/mask/pool are all
                        # per-block; scores here are O(0.1) so exp() is
                        # overflow-safe without the usual max shift)
                        tmp = rp2.tile([128, D], bft, tag="sc")
                        nc.vector.tensor_mul(tmp[:], h_nm[:, b, :],
                                             attrep[:])
                        nc.vector.tensor_reduce(
                            out=sc_nm[:, b:b + 1], in_=tmp[:],
                            axis=mybir.AxisListType.X,
                            op=mybir.AluOpType.add)
                        nc.tensor.matmul(
                            out=pall_mean[0:PPC, :],
                            lhsT=pone_cj[:, b * PPC:(b + 1) * PPC],
                            rhs=h_nm[:, b, :],
                            start=(b == 0), stop=(b == NBLK - 1),
                            skip_group_check=True)
                        nc.scalar.activation(exl[:, b:b + 1],
                                             sc_nm[:, b:b + 1], AF.Exp)
                        nc.vector.tensor_mul(exl[:, b:b + 1],
                                             exl[:, b:b + 1],
                                             lys_nm[:, b:b + 1])
                        nc.vector.tensor_scalar_mul(
                            pex_all[:, b, :],
                            pone[:, b * PPC:(b + 1) * PPC],
                            exl[:, b:b + 1])
                        nc.tensor.matmul(
                            out=pall_att[0:PPC, :],
                            lhsT=pex_all[:, b, :], rhs=h_nm[:, b, :],
                            start=(b == 0), stop=(b == NBLK - 1),
                            skip_group_check=True)
                        nc.tensor.matmul(
                            out=pall_den[0:PPC, :],
                            lhsT=pex_all[:, b, :], rhs=ones_bf[:],
                            start=(b == 0), stop=(b == NBLK - 1),
                            skip_group_check=True)

            if DBG_DUMP_H:
                for b in range(NTB):
                    nc.gpsimd.dma_start(
                        out_h[:, b * 128:(b + 1) * 128],
                        h_fm[:, b * 128:(b + 1) * 128])
            if DBG_DUMP_TB:
                nc.gpsimd.dma_start(
                    out_tb[:].rearrange("p (k f) -> p k f", k=NTB),
                    staging[:])

            if DBG_NO_READOUT:
                oz = rp2.tile([64, PPC], f32, tag="oz")
                nc.vector.tensor_copy(oz[:], h_nm[0:64, 0, 0:PPC])
                nc.gpsimd.dma_start(out_t[:], oz[:])

            if not DBG_NO_READOUT:
                # c_j = 1/(max(cnt,1)*sqrt(cnt+1e-6)) is folded into pone_cj
                # host-side, so the protein term comes out of its matmul
                # pre-scaled and the combine is one fused DVE op
                dg = gp.tile([PPC, 1], f32)
                nc.vector.tensor_scalar_max(dg[:], pall_den[0:PPC, :],
                                            1.0e-30)
                rden = gp.tile([PPC, 1], f32)
                nc.vector.reciprocal(rden[:], dg[:])

                lw = gp.tile([PPC, 128], f32)
                nc.vector.tensor_scalar_mul(lw[:], pall_att[0:PPC, :],
                                            rden[:])
                pre = gp.tile([PPC, 128], f32)
                nc.vector.tensor_add(pre[:], lw[:], pall_mean[0:PPC, :])
                ptp = ps_r.tile([128, 128], f32, tag="tr")
                nc.tensor.transpose(
                    out=ptp[:, 0:PPC], in_=pre[:],
                    identity=ident[0:PPC, 0:PPC])
                preT = gp.tile([128, PPC], f32)
                nc.vector.tensor_copy(preT[:], ptp[:, 0:PPC])
                pso = ps_r.tile([128, 128], f32, tag="tr")
                nc.tensor.matmul(
                    out=pso[0:64, 0:PPC], lhsT=outw[:], rhs=preT[:],
                    start=True, stop=True)
                osb = gp.tile([64, PPC], f32)
                nc.vector.tensor_scalar_add(osb[:], pso[0:64, 0:PPC],
                                            outb[:])
                nc.sync.dma_start(out_t[:], osb[:])

    nc.compile()
    return nc


# ---------------------------------------------------------------- entry

def kernel(**inputs):
    x = np.asarray(inputs["x"], np.float32)
    edge_index = np.asarray(inputs["edge_index"])
    batch = np.asarray(inputs["batch"])
    lysine_mask = np.asarray(inputs["lysine_mask"])
    conv_w = np.asarray(inputs["conv_w"], np.float32)
    conv_b = np.asarray(inputs["conv_b"], np.float32)
    att_w = np.asarray(inputs["att_w"], np.float32)
    out_w = np.asarray(inputs["out_w"], np.float32)
    out_b = np.asarray(inputs["out_b"], np.float32)

    per_core, LO_T, HI_T, NT = _host_prep(x, edge_index, batch, lysine_mask)

    convw = np.ascontiguousarray(
        np.concatenate([conv_w[i] for i in range(L)], axis=1)).astype(bf16)
    convb_pre = np.tile(
        np.concatenate([16.0 * conv_b[i] for i in range(L)]).astype(E4),
        (128, 1))
    shared = dict(
        convw=convw, convb_pre=convb_pre,
        attw_row=att_w.reshape(1, D).astype(np.float32),
        outw=out_w.astype(np.float32),
        outb=out_b.reshape(64, 1).astype(np.float32),
    )
    in_maps = []
    for c in range(NC):
        pc = per_core[c]
        in_maps.append({
            "x_t": pc["x_t"], "rowdst": pc["rowdst"],
            "iota_row": pc["iota_row"], "inv16": pc["inv16"],
            "s_part": pc["s_part"],
            "idx_lo": pc["idx_lo"], "idx_hi": pc["idx_hi"],
            "dis_nm": pc["dis_nm"],
            "pone": pc["pone"], "pone_cj": pc["pone_cj"],
            "lys_nm": pc["lys_nm"], **shared,
        })

    nc_prog = _build_program(LO_T, HI_T, NT)
    trace = os.environ.get("GCN_TRACE", "") == "1"
    res = run_bass_kernel_spmd(
        nc_prog, in_maps, core_ids=list(range(NC)), trace=trace)
    if trace:
        import kernel as _self
        _self.LAST_RESULT = res
        print("HW exec time:", res.exec_time_ns, "ns")
    out = np.concatenate(
        [np.asarray(res.results[c]["out_t"], np.float32).T for c in range(NC)],
        axis=0)
    return out



# revision 102
# speedup vs baseline: 1.0275x; 1.0056x over previous
"""GCN message-passing kernel for Trainium2, 8 NeuronCores (SPMD).

Strategy (graph-parallel, fp8 messages):
- Nodes are protein-contiguous, sharded across 8 cores at protein boundaries
  (16 proteins/core, padded to 6400 nodes/core). Within a core, nodes are
  bin-packed into 50 blocks of 128 balancing incoming-edge counts; slot
  s = blk*128 + pos maps to partition pos, chunk blk everywhere (h, dis,
  table, S, pooling) so aggregation blocks coincide with node chunks.
- Message table is fp8e4m3 at 256B row stride with a 128B payload
  (row = dis[src] * (h @ W)[src]); gathers use elem_size=128/elem_step=256
  (bass's %256 payload assert is bypassed via direct InstDMAGatherAnt
  construction - verified byte-exact on hardware), halving per-edge DMA
  cost vs bf16. Only real edges are gathered: the self-loop term is an
  identity matmul on the resident staging chunk, and the conv bias is a
  per-block diag(1/(16 dis)) @ (16 conv_b) matmul - both accumulate into
  the same PSUM group, so the gather stream is edges-only (NT=12 tiles
  per block instead of 13).
- Aggregation is node-major: acc[dst,feat] = S^T @ msgs with S a 0/1
  one-hot (exact in fp8) via DoubleRow fp8 matmuls (256 slots each);
  dis[dst] is applied EXACTLY by the relu epilogue's per-partition ACT
  scale. S rows are pure one-hots, so the first SGEN_BLOCKS blocks of S
  are generated on-chip (one DVE is_equal per block against an iota row,
  from a compact per-slot dst-position table) and only the rest load from
  DRAM - the DMA engines are the global bottleneck, DVE has slack.
- Pipeline: hi-stream gather chunks carry no dependency on the measured
  program's table write, so ring-depth many are prefetched to cover each
  layer's table-write + first-lo-descgen window; the table is written in
  halves so early wcasts stream out under the previous layer's gathers.
- h is kept bf16 both node-major (epilogue output; feeds readout) and
  feature-major (one PE transpose per chunk per layer; copies on Act
  while DVE generates S, on DVE afterwards).
- Readout is per-block in the last layer: scores via DVE mul+reduce
  against a broadcast att_w row, exp/mask immediately (denominator folded
  in at the end - softmax shift is skipped since scores are O(0.1)), and
  three accumulating pool matmuls (mean with host-folded 1/(n sqrt n),
  attention numerator, denominator); the final combine, transpose and
  projection run once per core after the block loop.
"""
import bisect
import os
import numpy as np
import ml_dtypes

DBG_LAYERS = int(os.environ.get("GCN_DBG_LAYERS", "4"))
DBG_NO_COLL = os.environ.get("GCN_DBG_NO_COLL", "") == "1"
DBG_NO_GATHER = os.environ.get("GCN_DBG_NO_GATHER", "") == "1"
DBG_NO_READOUT = os.environ.get("GCN_DBG_NO_READOUT", "") == "1"
DBG_DUMP_H = os.environ.get("GCN_DBG_DUMP_H", "") == "1"
DBG_DUMP_TB = os.environ.get("GCN_DBG_DUMP_TB", "") == "1"

import concourse.bacc as bacc
import concourse.tile as tile
import concourse.tile_utils as tile_utils
from concourse import mybir
from concourse.bass_utils import run_bass_kernel_spmd
from concourse.masks import make_identity

bf16 = ml_dtypes.bfloat16
E4 = ml_dtypes.float8_e4m3
AF = mybir.ActivationFunctionType

NC = 8
D = 128
L = 4
B = 128
PPC = B // NC          # proteins per core
NPAD = 6400            # padded nodes per core
NPADG = NC * NPAD      # global padded rows
NTB = NPAD // 128      # 50 chunks of 128 nodes == aggregation blocks
NBLK = NTB
LO_BOUND = 32000       # lo gather covers rows [0, 32000)
HI_BASE = 18560        # hi gather covers rows [18560, 51200): 32639 <= int16
GCH = 8192             # gather slots per dma_gather instruction (64 cols)
SGEN_BLOCKS = 28       # S blocks generated on-chip; the rest load from DRAM

f32 = mybir.dt.float32
bft = mybir.dt.bfloat16
fp8 = mybir.dt.float8e4
i16 = mybir.dt.int16


# ---------------------------------------------------------------- host prep

def _pack_idx(vals, slots):
    """int16 gather index layout: position i -> partition i%16, col i//16,
    replicated across the 128 partitions."""
    assert len(vals) == slots and slots % 16 == 0
    arr = np.asarray(vals, np.int16).reshape(slots // 16, 16).T  # [16, s//16]
    return np.ascontiguousarray(np.tile(arr, (8, 1)))


def _ceil128(x):
    return max(1, int(np.ceil(x / 128)))


def _host_prep(x, edge_index, batch, lysine_mask):
    N = x.shape[0]
    src = np.asarray(edge_index[0], np.int64)
    dst = np.asarray(edge_index[1], np.int64)
    batch = np.asarray(batch, np.int64)

    pcounts = np.bincount(batch, minlength=B)
    pstart = np.concatenate([[0], np.cumsum(pcounts)])
    cstart = pstart[np.arange(NC) * PPC]
    cend = pstart[(np.arange(NC) + 1) * PPC]
    ncore = cend - cstart
    assert ncore.max() <= NPAD - 1, f"core node count {ncore.max()} > {NPAD-1}"
    assert pcounts.max() <= 128 * NTB

    deg = np.bincount(dst, minlength=N).astype(np.float64) + 1.0
    dis = (1.0 / np.sqrt(deg)).astype(np.float32)
    core_of = np.searchsorted(cend, np.arange(N), side="right")

    # --- per-core node packing into NBLK blocks of 128, balancing in-slot
    # (in-edges + self) counts per block; (pos 127, blk 49) is reserved.
    blk = np.zeros(N, np.int64)
    pos = np.zeros(N, np.int64)
    for c in range(NC):
        nodes = np.arange(cstart[c], cend[c])
        tot = deg[nodes]
        order = np.argsort(-tot, kind="stable")
        caps = np.full(NBLK, 128, np.int64)
        caps[NBLK - 1] = 127
        loads = np.zeros(NBLK)
        cnts = np.zeros(NBLK, np.int64)
        for i in order:
            masked = np.where(cnts < caps, loads, np.inf)
            b = int(np.argmin(masked))
            blk[nodes[i]] = b
            # (pos 0, blk 49) is the reserved bias slot on every core
            pos[nodes[i]] = cnts[b] + (1 if b == NBLK - 1 else 0)
            cnts[b] += 1
            loads[b] += tot[i]
    slot = blk * 128 + pos                    # local pi slot
    grow = core_of * NPAD + pos * NTB + blk   # global table row

    # --- edge list: real edges only. Self-loops are applied on-chip via an
    # identity matmul on the staging chunk; the conv bias enters via a
    # per-block diagonal matmul (diag(1/(16 dis)) @ 16*conv_b), so there are
    # no pseudo-edges and every S row is a pure one-hot — which lets S be
    # GENERATED on-chip (DVE is_equal against an iota row) from a compact
    # per-slot dst-position table (rowdst) instead of a 10MB fp8 load.
    e_row = grow[src]
    e_core = core_of[dst]
    e_blk = blk[dst]
    e_col = pos[dst]

    cls = np.where(e_row < HI_BASE, 0,
                   np.where(e_row < LO_BOUND, 1, 2))
    key = e_core * NBLK + e_blk
    nl0 = np.bincount(key[cls == 0], minlength=NC * NBLK)
    nf = np.bincount(key[cls == 1], minlength=NC * NBLK)
    tot_cb = np.bincount(key, minlength=NC * NBLK)

    best = None
    for LO_T in range(_ceil128(nl0.max()), _ceil128(nl0.max()) + 4):
        lo_fill = np.minimum(LO_T * 128, nl0 + nf)
        HI_T = _ceil128((tot_cb - lo_fill).max())
        if best is None or LO_T + HI_T < best[0] + best[1]:
            best = (LO_T, HI_T)
    LO_T, HI_T = best
    NT = LO_T + HI_T

    iota_row = np.tile(np.arange(128, dtype=np.float32), (128, 1))
    per_core = []
    for c in range(NC):
        m = e_core == c
        rows_e, blk_e, col_e, cls_e = (
            e_row[m], e_blk[m], e_col[m], cls[m])
        order = np.lexsort((col_e, cls_e, blk_e))
        rows_e, blk_e, col_e, cls_e = (
            rows_e[order], blk_e[order], col_e[order], cls_e[order])
        bstart = np.searchsorted(blk_e, np.arange(NBLK))
        bend = np.searchsorted(blk_e, np.arange(NBLK), side="right")

        nodes = np.arange(cstart[c], cend[c])
        # dis / inv-dis in pi layout (pads -> 1 / 0)
        dis_nm = np.ones((128, NTB), np.float32)
        inv_nm = np.zeros((128, NTB), np.float32)
        dis_nm[pos[nodes], blk[nodes]] = dis[nodes]
        inv_nm[pos[nodes], blk[nodes]] = 1.0 / dis[nodes]

        lo_idx = np.zeros(NBLK * LO_T * 128, np.int64)
        hi_idx = np.zeros(NBLK * HI_T * 128, np.int64)  # already HI_BASE-offset
        # per-slot dst position, 255 for padding (never matches iota 0..127)
        rowdst = np.full((128, NBLK * NT), 255.0, np.float32)
        s_all = np.zeros((128, NBLK * NT * 128), np.float32)
        for b in range(NBLK):
            sl = slice(bstart[b], bend[b])
            r_b, c_b, k_b = rows_e[sl], col_e[sl], cls_e[sl]
            n = len(r_b)
            n0 = int((k_b == 0).sum())
            nfb = int((k_b == 1).sum())
            take = min(LO_T * 128 - n0, nfb)
            assert take >= 0, f"block lo overflow {n0} > {LO_T*128}"
            nlo = n0 + take
            nhi = n - nlo
            assert nhi <= HI_T * 128
            for stream, cnt, off, idxarr, base_t, ibase in (
                (0, nlo, 0, lo_idx, 0, 0),
                (1, nhi, nlo, hi_idx, LO_T, HI_BASE),
            ):
                if cnt == 0:
                    continue
                rr = r_b[off:off + cnt] - ibase
                cc = c_b[off:off + cnt]
                T = LO_T if stream == 0 else HI_T
                idxarr[b * T * 128: b * T * 128 + cnt] = rr
                k = np.arange(cnt)
                p = k % 128
                t = base_t + k // 128
                rowdst[p, b * NT + t] = cc
                s_all[p, (b * NT + t) * 128 + cc] = 1.0

        x_t = np.zeros((D, NPAD), np.float32)
        x_t[:, slot[nodes]] = np.asarray(x[nodes], np.float32).T

        lens = pcounts[c * PPC:(c + 1) * PPC]
        starts = np.concatenate([[0], np.cumsum(lens)])[:-1]
        q = np.arange(ncore[c])
        pj = np.searchsorted(starts, q, side="right") - 1
        pone = np.zeros((128, NTB * PPC), bf16)
        pone[pos[nodes], blk[nodes] * PPC + pj] = 1.0
        lens_f = np.asarray(lens, np.float64)
        cj = (1.0 / (np.maximum(lens_f, 1.0) * np.sqrt(lens_f + 1e-6)))
        pone_cj = (pone.astype(np.float32)
                   * np.tile(cj, NTB)[None, :]).astype(bf16)
        lys_nm = np.zeros((128, NTB), np.float32)
        lys_nm[pos[nodes], blk[nodes]] = np.asarray(
            lysine_mask[nodes], np.float32)

        per_core.append(dict(
            x_t=x_t.astype(bf16),
            rowdst=rowdst,
            iota_row=iota_row,
            inv16=inv_nm / 16.0,
            s_part=np.ascontiguousarray(
                s_all[:, SGEN_BLOCKS * NT * 128:]).astype(E4),
            idx_lo=_pack_idx(lo_idx, NBLK * LO_T * 128),
            idx_hi=_pack_idx(hi_idx, NBLK * HI_T * 128),
            dis_nm=dis_nm,
            pone=pone,
            pone_cj=pone_cj,
            lys_nm=lys_nm,
        ))
    return per_core, LO_T, HI_T, NT


# ---------------------------------------------------------------- program

def _dma_gather_128(nc, out_ap, in_ap, idxs_ap, num_idxs):
    """dma_gather with a 128B payload on a 256B-stride table (elem_size=128
    fp8, elem_step=256). Bypasses bass's %256 payload assert; verified
    byte-exact on hardware."""
    g = nc.gpsimd
    _in_ap = g.lower_ap_dma(in_ap, for_custom_bir_dma=True)
    _idxs_ap = g.lower_ap(idxs_ap)
    _out_ap = g.lower_ap(out_ap)
    return g.add_instruction(mybir.InstDMAGatherAnt(
        name=g.bass.get_next_instruction_name(),
        ins=[*_in_ap, _idxs_ap, g.lower_val_access(g.to_reg(num_idxs))],
        outs=[_out_ap],
        transpose=False, num_idxs=num_idxs, elem_size=128,
        stride_bytes_256=1, gen_mode=0, single_packet=False,
        queue_num=0, sbuf_tokens_per_rank=0, sbuf_free_dim_per_rank=0,
        sbuf_free_dim_pad_per_rank=0, sbuf_byte_offset=0))


def _build_program(LO_T, HI_T, NT):
    tile_utils.max_sbuf_usage = 208 * 1024
    nc = bacc.Bacc("TRN2", target_bir_lowering=False, num_devices=NC,
                   num_swdge_queues=2)

    din = {}
    for name, shape, dt in [
        ("x_t", [D, NPAD], bft),
        ("rowdst", [128, NBLK * NT], f32),
        ("iota_row", [128, 128], f32),
        ("inv16", [128, NTB], f32),
        ("s_part", [128, (NBLK - SGEN_BLOCKS) * NT * 128], fp8),
        ("idx_lo", [128, NBLK * LO_T * 8], i16),
        ("idx_hi", [128, NBLK * HI_T * 8], i16),
        ("dis_nm", [128, NTB], f32),
        ("pone", [128, NTB * PPC], bft),
        ("pone_cj", [128, NTB * PPC], bft),
        ("lys_nm", [128, NTB], f32),
        ("convw", [D, L * D], bft),
        ("convb_pre", [128, L * D], fp8),
        ("attw_row", [1, D], f32),
        ("outw", [D, 64], f32),
        ("outb", [64, 1], f32),
    ]:
        din[name] = nc.dram_tensor(name, shape, dt, kind="ExternalInput")
    out_t = nc.dram_tensor("out_t", [64, PPC], f32, kind="ExternalOutput")
    out_h = None
    if DBG_DUMP_H:
        out_h = nc.dram_tensor("out_h", [128, NPAD], bft,
                               kind="ExternalOutput")
    out_tb = None
    if DBG_DUMP_TB:
        out_tb = nc.dram_tensor("out_tb", [128, NTB * 128], fp8,
                                kind="ExternalOutput")

    LO_SLOTS = NBLK * LO_T * 128
    HI_SLOTS = NBLK * HI_T * 128

    with tile.TileContext(nc) as tc:
        with (
            tc.tile_pool(name="glob", bufs=1) as gp,
            tc.tile_pool(name="dram", bufs=1, space="DRAM") as dram,
            tc.tile_pool(name="msgs", bufs=4) as mp,
            tc.tile_pool(name="r2", bufs=2) as rp2,
            tc.tile_pool(name="ps_w", bufs=2, space="PSUM") as ps_w,
            tc.tile_pool(name="ps_agg", bufs=3, space="PSUM") as ps_agg,
            tc.tile_pool(name="ps_tr", bufs=1, space="PSUM") as ps_tr,
            tc.tile_pool(name="ps_r", bufs=1, space="PSUM") as ps_r,
            tc.tile_pool(name="ps_p", bufs=1, space="PSUM") as ps_p,
        ):
            # resident SBUF state
            h_fm = gp.tile([D, NPAD], bft, name="h_fm")
            nc.sync.dma_start(h_fm[:], din["x_t"][:])
            h_nm = gp.tile([128, NTB, 128], bft, name="h_nm")
            staging = gp.tile([128, NTB, 128], fp8, name="staging")
            s_sb = gp.tile([128, NBLK * NT * 128], fp8, name="s_sb")
            # idx_hi loads first (it gates the first hi-gather desc-gen),
            # then the wcast deps (convw/dis/convb), then the rest
            idx_hi = gp.tile([128, HI_SLOTS // 16], i16)
            nc.sync.dma_start(idx_hi[:], din["idx_hi"][:])
            convw = gp.tile([D, L * D], bft)
            nc.sync.dma_start(convw[:], din["convw"][:])
            dis_nm = gp.tile([128, NTB], f32)
            nc.sync.dma_start(dis_nm[:], din["dis_nm"][:])
            convb_pre = gp.tile([128, L * D], fp8)
            nc.sync.dma_start(convb_pre[:], din["convb_pre"][:])
            idx_lo = gp.tile([128, LO_SLOTS // 16], i16)
            nc.sync.dma_start(idx_lo[:], din["idx_lo"][:])
            rowdst = gp.tile([128, NBLK * NT], f32)
            nc.sync.dma_start(rowdst[:], din["rowdst"][:])
            iota_row = gp.tile([128, 128], f32)
            nc.sync.dma_start(iota_row[:], din["iota_row"][:])
            inv16 = gp.tile([128, NTB], f32)
            nc.sync.dma_start(inv16[:], din["inv16"][:])
            pone = gp.tile([128, NTB * PPC], bft)
            nc.sync.dma_start(pone[:], din["pone"][:])
            lys_nm = gp.tile([128, NTB], f32)
            nc.sync.dma_start(lys_nm[:], din["lys_nm"][:])
            pone_cj = gp.tile([128, NTB * PPC], bft)
            nc.sync.dma_start(pone_cj[:], din["pone_cj"][:])
            attw = gp.tile([1, D], f32)
            nc.sync.dma_start(attw[:], din["attw_row"][:])
            outw = gp.tile([D, 64], f32)
            nc.sync.dma_start(outw[:], din["outw"][:])
            outb = gp.tile([64, 1], f32)
            nc.sync.dma_start(outb[:], din["outb"][:])
            stripe = dram.tile([NPAD, 256], fp8)
            hws_full = dram.tile([NPADG, 256], fp8)
            tident = gp.tile([128, 128], bft)
            make_identity(nc, tident[:])
            ident = gp.tile([128, 128], f32)
            make_identity(nc, ident[:])
            ident8 = gp.tile([128, 128], fp8)
            make_identity(nc, ident8[:])
            ones_r = gp.tile([1, 128], f32)
            nc.vector.memset(ones_r[:], 1.0)
            ones_f = gp.tile([128, 1], f32)
            nc.vector.memset(ones_f[:], 1.0)
            ones_bf = gp.tile([128, 1], bft)
            nc.vector.memset(ones_bf[:], 1.0)

            # att_w broadcast to all partitions (ones outer product)
            psat = ps_r.tile([128, D], f32, tag="tr")
            nc.tensor.matmul(out=psat[:], lhsT=ones_r[:],
                             rhs=attw[:], start=True, stop=True)
            attrep = gp.tile([128, D], bft)
            nc.vector.tensor_copy(attrep[:], psat[:])
            sc_nm = gp.tile([128, NTB], f32)
            exl = gp.tile([128, NTB], f32)
            pex_all = gp.tile([128, NTB, PPC], bft)

            # per-block diag(1/(16 dis[dst])) in fp8: the bias matmul's lhsT
            # (same quantization as the old in-S bias column); filled
            # per-block alongside the S generation / load below
            diag8 = gp.tile([128, NTB, 128], fp8)

            def emit_sgen(b):
                # S block b: one-hot rows from per-slot dst positions via a
                # single DVE is_equal (iota row tiled along tiles; rowdst
                # broadcast along the 128 dst columns). Padding slots carry
                # 255 and generate all-zero rows. ~1.66us per block on DVE,
                # which only keeps ahead of the layer-0 aggregation pace for
                # the first SGEN_BLOCKS blocks — the rest load from DRAM.
                nc.vector.tensor_tensor(
                    out=s_sb[:, b * NT * 128:(b + 1) * NT * 128].rearrange(
                        "p (t c) -> p t c", t=NT),
                    in0=iota_row[:].rearrange(
                        "p (o c) -> p o c", o=1).broadcast_to([128, NT, 128]),
                    in1=rowdst[:, b * NT:(b + 1) * NT].rearrange(
                        "p (t o) -> p t o", o=1).broadcast_to([128, NT, 128]),
                    op=mybir.AluOpType.is_equal)
                nc.vector.tensor_scalar_mul(
                    diag8[:, b, :], ident8[:], inv16[:, b:b + 1])

            def emit_wcast(layer, b, dve=False):
                # table chunk: staging[:, b, :] = fp8(dis * (h @ W)),
                # node-major via out = h_fm_chunk^T @ W
                pw = ps_w.tile([128, D], f32, tag="wmm")
                nc.tensor.matmul(
                    out=pw[:],
                    lhsT=h_fm[:, b * 128:(b + 1) * 128],
                    rhs=convw[:, layer * D:(layer + 1) * D],
                    start=True, stop=True)
                if dve:
                    nc.vector.tensor_scalar_mul(
                        staging[:, b, :], pw[:], dis_nm[:, b:b + 1])
                else:
                    nc.scalar.activation(
                        staging[:, b, :], pw[:], AF.Copy,
                        scale=dis_nm[:, b:b + 1])

            SGEN_AHEAD = 10
            # layer 0's chain alternates Act/DVE epilogues: it gates the
            # first table write, and DVE's S generation only starts once
            # rowdst lands
            for b in range(NTB):
                emit_wcast(0, b, dve=(b % 2 == 1))
            for b in range(min(SGEN_AHEAD, SGEN_BLOCKS)):
                emit_sgen(b)
            for b in range(SGEN_BLOCKS, NBLK):
                nc.vector.tensor_scalar_mul(
                    diag8[:, b, :], ident8[:], inv16[:, b:b + 1])
            # S for the later half of the loaded blocks arrives early on the
            # Act queue; the first piece is issued behind layer 0's table
            # write on the SP queue, where it fills the DMA engines during
            # the first lo gather's desc-gen window
            SG0 = SGEN_BLOCKS * NT * 128
            SGM = ((SGEN_BLOCKS + NBLK) // 2) * NT * 128
            nc.scalar.dma_start(s_sb[:, SGM:],
                                din["s_part"][:, SGM - SG0:])

            pall_mean = None
            pall_att = None
            for layer in range(DBG_LAYERS):
                last = layer == DBG_LAYERS - 1
                # table write in pieces so early-chunk payloads stream out
                # while later wcasts still run — the next layer's lo gathers
                # wait on all of them
                if DBG_NO_COLL:
                    # collective stand-in: write the staged payload straight
                    # into this core's own region of the table (same local
                    # DMA work as the real path's stripe write)
                    tpm = hws_full[0:NPAD, 0:128].rearrange(
                        "(p k) f -> p k f", k=NTB)
                else:
                    tpm = stripe[:, 0:128].rearrange("(p k) f -> p k f", k=NTB)
                for k0, k1 in ((0, NTB // 2), (NTB // 2, NTB)):
                    nc.sync.dma_start(tpm[:, k0:k1, :], staging[:, k0:k1, :])
                if not DBG_NO_COLL:
                    nc.gpsimd.collective_compute(
                        "AllGather", mybir.AluOpType.bypass,
                        replica_groups=[list(range(NC))],
                        ins=[stripe.opt()], outs=[hws_full.opt()])

                # gathers issued lazily in consumption order; aggregate
                # via DoubleRow fp8 matmuls; relu epilogue with exact
                # dis[dst] as the ACT per-partition scale. In the last layer
                # the trailing chunks are split finer so the final blocks'
                # readout chains aren't backlogged behind one wide transfer.
                lo_chunks, hi_chunks = {}, {}

                def mk_plan(slots):
                    starts, s = [], 0
                    while s < slots:
                        starts.append(s)
                        s += min(GCH, slots - s)
                    return starts

                lo_plan = mk_plan(LO_SLOTS)
                hi_plan = mk_plan(HI_SLOTS)

                def col_chunk(plan, col):
                    # chunk id + col offset for tile-column `col`
                    ci = bisect.bisect_right(plan, col * 128) - 1
                    return ci, col - plan[ci] // 128

                def get_chunk(done, ci, plan, slots, idx, base_hi, tg):
                    if ci not in done:
                        s0 = plan[ci]
                        n = (plan[ci + 1] if ci + 1 < len(plan)
                             else slots) - s0
                        m = mp.tile([128, GCH // 128, 128], fp8, tag=tg,
                                    bufs=4)
                        if DBG_NO_GATHER:
                            nc.vector.memset(m[:], 0.0)
                        else:
                            src_ap = (hws_full[HI_BASE:, 0:128] if base_hi
                                      else hws_full[:, 0:128])
                            _dma_gather_128(
                                nc, m[:, : n // 128, :], src_ap,
                                idx[:, s0 // 16:(s0 + n) // 16], n)
                        done[ci] = m
                    return done[ci]

                # prefetch: hi chunks have no dep on this core's table write
                # in the measured program, so they can fill the DMA engines
                # across the table-write + first-lo-descgen window. Exactly
                # ring-depth many go ahead of lo chunk 0 (one more would
                # WAR-wait on hi chunk 0's consumers, which need lo chunk 0
                # -> deadlock on the in-order Pool queue).
                for ci in range(min(4, len(hi_plan))):
                    get_chunk(hi_chunks, ci, hi_plan, HI_SLOTS, idx_hi,
                              True, "mhi")
                if layer == 0:
                    # the s_part head piece rides the Pool queue between the
                    # hi prefetches and lo chunk 0, pinned behind the table
                    # write by a 128B table read whose output the load then
                    # overwrites (WAW keeps the scheduler from hoisting it):
                    # its transfer covers lo chunk 0's desc-gen window
                    nc.gpsimd.dma_start(s_sb[0:1, SG0:SG0 + 128],
                                        hws_full[0:1, 0:128])
                    nc.gpsimd.dma_start(s_sb[:, SG0:SGM],
                                        din["s_part"][:, 0:SGM - SG0])
                get_chunk(lo_chunks, 0, lo_plan, LO_SLOTS, idx_lo,
                          False, "mlo")

                if last and not DBG_NO_READOUT:
                    # reuse the idle "wmm" ring (no W matmuls in last layer)
                    pall_mean = ps_w.tile([128, D], f32, tag="wmm")
                    pall_att = ps_w.tile([128, D], f32, tag="wmm")
                    pall_den = ps_p.tile([128, 1], f32, tag="pden")
                for b in range(NBLK):
                    acc = ps_agg.tile([128, D], f32, tag="agg")
                    # self-loop term: staging row v already holds
                    # fp8(dis[v]*(h@W)[v]); identity matmul adds it to acc,
                    # the relu epilogue's dis scale makes it dis^2*(h@W).
                    nc.tensor.matmul(
                        out=acc[:], lhsT=ident8[:], rhs=staging[:, b, :],
                        start=True, stop=False)
                    # conv bias: diag(1/(16 dis)) @ (16 conv_b replicated)
                    # -> inv_dis*conv_b, the epilogue dis scale -> conv_b
                    nc.tensor.matmul(
                        out=acc[:], lhsT=diag8[:, b, :],
                        rhs=convb_pre[:, layer * D:(layer + 1) * D],
                        start=False, stop=False)
                    # plan matmuls: DoubleRow pairs where chunk-aligned,
                    # plain fp8 matmuls for odd tails / chunk straddles
                    ops = []
                    for T, base_t, st, plan in ((LO_T, 0, 0, lo_plan),
                                                (HI_T, LO_T, 1, hi_plan)):
                        t = 0
                        while t < T:
                            col = b * T + t
                            if (t + 1 < T and col_chunk(plan, col)[0]
                                    == col_chunk(plan, col + 1)[0]):
                                ops.append((st, T, base_t, t, 2))
                                t += 2
                            else:
                                ops.append((st, T, base_t, t, 1))
                                t += 1
                    for k, (st, T, base_t, t, w) in enumerate(ops):
                        col = b * T + t
                        if st == 0:
                            ci, cc = col_chunk(lo_plan, col)
                            mm = get_chunk(lo_chunks, ci, lo_plan, LO_SLOTS,
                                           idx_lo, False, "mlo")
                        else:
                            ci, cc = col_chunk(hi_plan, col)
                            mm = get_chunk(hi_chunks, ci, hi_plan, HI_SLOTS,
                                           idx_hi, True, "mhi")
                        sc0 = (b * NT + base_t + t) * 128
                        if w == 2:
                            nc.tensor.matmul(
                                out=acc[:],
                                lhsT=s_sb[:, sc0:sc0 + 256].rearrange(
                                    "p (i d) -> p i d", i=2),
                                rhs=mm[:, cc:cc + 2, :],
                                start=False, stop=(k == len(ops) - 1),
                                perf_mode=mybir.MatmulPerfMode.DoubleRow)
                        else:
                            nc.tensor.matmul(
                                out=acc[:],
                                lhsT=s_sb[:, sc0:sc0 + 128],
                                rhs=mm[:, cc, :],
                                start=False, stop=(k == len(ops) - 1))
                    nc.scalar.activation(
                        h_nm[:, b, :], acc[:], AF.Relu,
                        scale=dis_nm[:, b:b + 1])
                    if layer == 0 and b + SGEN_AHEAD < SGEN_BLOCKS:
                        emit_sgen(b + SGEN_AHEAD)
                    if not last or DBG_DUMP_H:
                        pt = ps_tr.tile([128, 128], bft, tag="ptr")
                        nc.tensor.transpose(
                            out=pt[:], in_=h_nm[:, b, :],
                            identity=tident[:])
                        # PSUM->SBUF copy: on Act while layer 0's DVE is
                        # saturated by the S generation, on DVE otherwise
                        if layer == 0 and b < SGEN_BLOCKS:
                            nc.scalar.activation(
                                h_fm[:, b * 128:(b + 1) * 128], pt[:],
                                AF.Copy)
                        else:
                            nc.vector.tensor_copy(
                                h_fm[:, b * 128:(b + 1) * 128], pt[:])
                    if not last:
                        # next layer's table chunk, pipelined under this
                        # layer's gather phase
                        emit_wcast(layer + 1, b)
                    elif not DBG_NO_READOUT:
                        # readout pieces that only need h_nm[b]: scores
                        # (DVE mul+reduce), the mean-pool matmul, and the
                        # attention-pool matmul (softmax denominator is
                        # folded in at the end, so exp/mask/pool are all
                        # per-block; scores here are O(0.1) so exp() is
                        # overflow-safe without the usual max shift)
                        tmp = rp2.tile([128, D], bft, tag="sc")
                        nc.vector.tensor_mul(tmp[:], h_nm[:, b, :],
                                             attrep[:])
                        nc.vector.tensor_reduce(
                            out=sc_nm[:, b:b + 1], in_=tmp[:],
                            axis=mybir.AxisListType.X,
                            op=mybir.AluOpType.add)
                        nc.tensor.matmul(
                            out=pall_mean[0:PPC, :],
                            lhsT=pone_cj[:, b * PPC:(b + 1) * PPC],
                            rhs=h_nm[:, b, :],
                            start=(b == 0), stop=(b == NBLK - 1),
                            skip_group_check=True)
                        nc.scalar.activation(exl[:, b:b + 1],
                                             sc_nm[:, b:b + 1], AF.Exp)
                        nc.vector.tensor_mul(exl[:, b:b + 1],
                                             exl[:, b:b + 1],
                                             lys_nm[:, b:b + 1])
                        nc.vector.tensor_scalar_mul(
                            pex_all[:, b, :],
                            pone[:, b * PPC:(b + 1) * PPC],
                            exl[:, b:b + 1])
                        nc.tensor.matmul(
                            out=pall_att[0:PPC, :],
                            lhsT=pex_all[:, b, :], rhs=h_nm[:, b, :],
                            start=(b == 0), stop=(b == NBLK - 1),
                            skip_group_check=True)
                        nc.tensor.matmul(
                            out=pall_den[0:PPC, :],
                            lhsT=pex_all[:, b, :], rhs=ones_bf[:],
                            start=(b == 0), stop=(b == NBLK - 1),
                            skip_group_check=True)

            if DBG_DUMP_H:
                for b in range(NTB):
                    nc.gpsimd.dma_start(
                        out_h[:, b * 128:(b + 1) * 128],
                        h_fm[:, b * 128:(b + 1) * 128])
            if DBG_DUMP_TB:
                nc.gpsimd.dma_start(
                    out_tb[:].rearrange("p (k f) -> p k f", k=NTB),
                    staging[:])

            if DBG_NO_READOUT:
                oz = rp2.tile([64, PPC], f32, tag="oz")
                nc.vector.tensor_copy(oz[:], h_nm[0:64, 0, 0:PPC])
                nc.gpsimd.dma_start(out_t[:], oz[:])

            if not DBG_NO_READOUT:
                # c_j = 1/(max(cnt,1)*sqrt(cnt+1e-6)) is folded into pone_cj
                # host-side, so the protein term comes out of its matmul
                # pre-scaled and the combine is one fused DVE op
                dg = gp.tile([PPC, 1], f32)
                nc.vector.tensor_scalar_max(dg[:], pall_den[0:PPC, :],
                                            1.0e-30)
                rden = gp.tile([PPC, 1], f32)
                nc.vector.reciprocal(rden[:], dg[:])

                lw = gp.tile([PPC, 128], f32)
                nc.vector.tensor_scalar_mul(lw[:], pall_att[0:PPC, :],
                                            rden[:])
                pre = gp.tile([PPC, 128], f32)
                nc.vector.tensor_add(pre[:], lw[:], pall_mean[0:PPC, :])
                ptp = ps_r.tile([128, 128], f32, tag="tr")
                nc.tensor.transpose(
                    out=ptp[:, 0:PPC], in_=pre[:],
                    identity=ident[0:PPC, 0:PPC])
                preT = gp.tile([128, PPC], f32)
                nc.vector.tensor_copy(preT[:], ptp[:, 0:PPC])
                pso = ps_r.tile([128, 128], f32, tag="tr")
                nc.tensor.matmul(
                    out=pso[0:64, 0:PPC], lhsT=outw[:], rhs=preT[:],
                    start=True, stop=True)
                osb = gp.tile([64, PPC], f32)
                nc.vector.tensor_scalar_add(osb[:], pso[0:64, 0:PPC],
                                            outb[:])
                nc.sync.dma_start(out_t[:], osb[:])

    nc.compile()
    return nc


# ---------------------------------------------------------------- entry

def kernel(**inputs):
    x = np.asarray(inputs["x"], np.float32)
    edge_index = np.asarray(inputs["edge_index"])
    batch = np.asarray(inputs["batch"])
    lysine_mask = np.asarray(inputs["lysine_mask"])
    conv_w = np.asarray(inputs["conv_w"], np.float32)
    conv_b = np.asarray(inputs["conv_b"], np.float32)
    att_w = np.asarray(inputs["att_w"], np.float32)
    out_w = np.asarray(inputs["out_w"], np.float32)
    out_b = np.asarray(inputs["out_b"], np.float32)

    per_core, LO_T, HI_T, NT = _host_prep(x, edge_index, batch, lysine_mask)

    convw = np.ascontiguousarray(
        np.concatenate([conv_w[i] for i in range(L)], axis=1)).astype(bf16)
    convb_pre = np.tile(
        np.concatenate([16.0 * conv_b[i] for i in range(L)]).astype(E4),
        (128, 1))
    shared = dict(
        convw=convw, convb_pre=convb_pre,
        attw_row=att_w.reshape(1, D).astype(np.float32),
        outw=out_w.astype(np.float32),
        outb=out_b.reshape(64, 1).astype(np.float32),
    )
    in_maps = []
    for c in range(NC):
        pc = per_core[c]
        in_maps.append({
            "x_t": pc["x_t"], "rowdst": pc["rowdst"],
            "iota_row": pc["iota_row"], "inv16": pc["inv16"],
            "s_part": pc["s_part"],
            "idx_lo": pc["idx_lo"], "idx_hi": pc["idx_hi"],
            "dis_nm": pc["dis_nm"],
            "pone": pc["pone"], "pone_cj": pc["pone_cj"],
            "lys_nm": pc["lys_nm"], **shared,
        })

    nc_prog = _build_program(LO_T, HI_T, NT)
    trace = os.environ.get("GCN_TRACE", "") == "1"
    res = run_bass_kernel_spmd(
        nc_prog, in_maps, core_ids=list(range(NC)), trace=trace)
    if trace:
        import kernel as _self
        _self.LAST_RESULT = res
        print("HW exec time:", res.exec_time_ns, "ns")
    out = np.concatenate(
        [np.asarray(res.results[c]["out_t"], np.float32).T for c in range(NC)],
        axis=0)
    return out

